# revision 66
# baseline (speedup 1.0000x reference)
"""Causal self-attention kernel for 8 TRN2 NeuronCores.

Problem: B=4, T=2048, C=1024, H=16 heads, D=64 (fp32 in/out).

Sharding: 8 cores = 4 batch entries x 2 head-groups (8 heads each).
Each core computes, for its (batch b, head-group hg):
    qkv slice -> flash-style causal attention (no-max softmax) -> partial
    projection y_part = attn_out @ W_proj[rows of its heads].
Host sums the two partial projections per batch entry.

Precision strategy (row-split): query rows t<512 (tq-group g=0) follow the
bf16 path everywhere (errors there don't average over keys, so they must
stay accurate); rows t>=512 use fp8e4 DoubleRow matmuls (0.5 cyc/row, 2x
effective contraction) for qkv-gen, att@V and the projection, plus fp8
attention weights produced either by ACT exp (fp8 out, shifted by -SHIFT to
fit e4m3's 240 max) or by a one-op DVE "Schraudolph" exp: psS already holds
S*log2e (k_t is pre-scaled by log2e), so uint8(round(min(psS + S0, 119)))
bit-cast to fp8e4 IS exp(s-SHIFT) to ~3%; uint8 saturation-at-0 provides
the lower clamp. The softmax ratio cancels the shift exactly.

fp8 operands are pre-scaled into e4m3's sweet spot (W by 32, y2 by 4) and
the inverse scales are folded into existing DVE copies, so they are free.

On-device layout:
  - QK^T contracts 64 dims/head (bf16, psS = S*log2e); exp -> pT (g0: bf16;
    g>=1: fp8 in chunk-PAIRED tiles [128, 2, 2h, 512]).
  - att@V g0: per-(chunk,head,block) solo bf16 matmuls (64-col moving);
    g>=1: DoubleRow over chunk pairs (stationary [tk,2,128tq] pT-pair,
    moving [tk,2,64] v-pair), deferred by one pair so the in-order PE queue
    never blocks the next pair's QK^T behind the exp wait.
  - psY is exactly ONE PSUM bank per group ([128,2h,4j,64]); all 16 groups'
    softmax denominators accumulate into a single shared PSUM bank via
    start=False 1-col matmuls (bank memset once; start_tensor_calc would
    wipe the whole bank, so it is never used there).
  - projection: chunks 0-3 bf16 (4-pair contraction); chunks 4-15 DoubleRow
    over pair-pairs from yT2_8 (fp8 copy of the transposed normalized y2).
  - diag masking via tri multiply on DVE (bf16 for g0, fp8 late;
    gpsimd masking loses ~4.7us: Pool queues them behind SWDGE desc-gen).
Software-pipelined emission: deadline-ordered work queue of qkv/proj chunks
interleaved into the ACT/DVE-paced attention loop, as before.
PSUM budget: 2 util + 2x2 psS + 1 psY + 1 den = 8 banks.
TimelineSim: 203.5us (baseline bf16 kernel: 241.5us); rel err 1.08e-2.
"""

import numpy as np
import ml_dtypes
import sys

sys.path.insert(0, "/opt/trn_rl_repo")

import concourse.bass as bass
import concourse.mybir as mybir
import concourse.tile as tile
from concourse import bacc
from concourse.bass_utils import run_bass_kernel_spmd

BF = mybir.dt.bfloat16
F32 = mybir.dt.float32
FP8 = mybir.dt.float8e4
U8 = mybir.dt.uint8
AF = mybir.ActivationFunctionType
DR = mybir.MatmulPerfMode.DoubleRow

B, T, C = 4, 2048, 1024
H, D = 16, 64
N_CORES = 8
HEADS_PER_CORE = 8          # 4 pairs
PAIRS = 4
TC = T // 128               # 16 t-chunks of 128
TG = T // 512               # 4 t-groups of 512
CT = C // 128               # 8 contraction tiles
CIP = CT // 2               # 4 contraction-tile pairs (DoubleRow)
TL = T - 512                # late-row span (t >= 512)

LOG2E = 1.4426950408889634
KAPPA = 1.0 / (8.0 * LOG2E)     # activation scale: exp(psS*KAPPA) = exp(s~)
SHIFT = 3.2                     # p~ = exp(s~ - SHIFT) keeps e4m3 finite
DELTA = 0.32                    # schraudolph interp-error centering
S0 = 56.0 - DELTA - SHIFT * 8.0 * LOG2E
WS = 32.0                       # fp8 weight pre-scale
YS = 4.0                        # fp8 y2 pre-scale
ZINV = 1.0 / (WS * YS)          # late proj PSUM descale

_compiled = None
_DEBUG = False
_EXP_MODE = "mix"   # "act" | "schr" | "mix"
_SCHR_MOD = 8       # in mix mode: chunk is schraudolph iff (c+4g+p)%MOD==MOD-1
_Y2_POOL = False    # fused y2 normalize on gpsimd instead of DVE
_QK_COPY_ACT = 0    # 0=DVE; 1=late k copies on ACT; 2=late q+k copies on ACT


def _build():
    nc = bacc.Bacc("TRN2", target_bir_lowering=False)

    xT = nc.declare_dram_parameter("xT", [C, 512], BF, isOutput=False)
    x8 = nc.declare_dram_parameter("x8", [C, TL], FP8, isOutput=False)
    wq = nc.declare_dram_parameter("wq", [C, 512], BF, isOutput=False)
    wk = nc.declare_dram_parameter("wk", [C, 512], BF, isOutput=False)
    wv = nc.declare_dram_parameter("wv", [C, 512], BF, isOutput=False)
    w8q = nc.declare_dram_parameter("w8q", [C, 512], FP8, isOutput=False)
    w8k = nc.declare_dram_parameter("w8k", [C, 512], FP8, isOutput=False)
    w8v = nc.declare_dram_parameter("w8v", [C, 512], FP8, isOutput=False)
    wp = nc.declare_dram_parameter("wp", [512, C], BF, isOutput=False)
    wp8 = nc.declare_dram_parameter("wp8", [512, C], FP8, isOutput=False)
    tri = nc.declare_dram_parameter("tri", [128, 128], BF, isOutput=False)
    y = nc.declare_dram_parameter("y", [T, C], F32, isOutput=True)

    with tile.TileContext(nc) as tc:
        with (
            tc.tile_pool(name="const", bufs=1) as cpool,
            tc.tile_pool(name="small", bufs=2) as spool,
            tc.tile_pool(name="ps_util", bufs=2, space="PSUM") as ps_util,
            tc.tile_pool(name="ps_s", bufs=2, space="PSUM") as ps_s,
            tc.tile_pool(name="ps_y", bufs=1, space="PSUM") as ps_y,
            tc.tile_pool(name="ps_den", bufs=1, space="PSUM") as ps_den,
        ):
            # softmax denominators for all 16 (pair, group)s live in one
            # shared PSUM bank: zeroed once, then accumulated into by
            # start=False 1-col matmuls (never start_tensor_calc'd, so no
            # bank wipes).  This keeps each psY to exactly one bank.
            psDen = ps_den.tile([128, 16, 2, 4], F32, name="psDen", tag="den")
            nc.vector.memset(psDen[:], 0.0)
            # ---------------- constant loads ----------------
            xT_t = cpool.tile([128, CT, 512], BF, name="xT_t", tag="xT_t")
            x8_t = cpool.tile([128, CT, TL], FP8, name="x8_t", tag="x8_t")
            wv_t = cpool.tile([128, CT, 512], BF, name="wv_t", tag="wv_t")
            wq_t = cpool.tile([128, CT, 512], BF, name="wq_t", tag="wq_t")
            wk_t = cpool.tile([128, CT, 512], BF, name="wk_t", tag="wk_t")
            w8v_t = cpool.tile([128, CIP, 2, 512], FP8, name="w8v_t", tag="w8v_t")
            w8q_t = cpool.tile([128, CIP, 2, 512], FP8, name="w8q_t", tag="w8q_t")
            w8k_t = cpool.tile([128, CIP, 2, 512], FP8, name="w8k_t", tag="w8k_t")
            tri_t = cpool.tile([128, 128], BF, name="tri_t", tag="tri_t")

            xT_r = xT.ap().rearrange("(ct p) t -> p ct t", p=128)
            nc.sync.dma_start(wv_t[:], wv.ap().rearrange("(ct p) d -> p ct d", p=128))
            nc.sync.dma_start(xT_t[:, :, 0:128], xT_r[:, :, 0:128])
            nc.sync.dma_start(xT_t[:, :, 128:512], xT_r[:, :, 128:512])
            nc.sync.dma_start(wk_t[:], wk.ap().rearrange("(ct p) d -> p ct d", p=128))
            nc.sync.dma_start(wq_t[:], wq.ap().rearrange("(ct p) d -> p ct d", p=128))
            nc.sync.dma_start(tri_t[:], tri.ap())
            nc.sync.dma_start(w8v_t[:],
                              w8v.ap().rearrange("(cp s p) d -> p cp s d", p=128, s=2))
            nc.sync.dma_start(x8_t[:], x8.ap().rearrange("(ct p) t -> p ct t", p=128))
            nc.sync.dma_start(w8k_t[:],
                              w8k.ap().rearrange("(cp s p) d -> p cp s d", p=128, s=2))
            nc.sync.dma_start(w8q_t[:],
                              w8q.ap().rearrange("(cp s p) d -> p cp s d", p=128, s=2))
            wp_t = cpool.tile([128, PAIRS, C], BF, name="wp_t", tag="wp_t")
            nc.sync.dma_start(wp_t[:], wp.ap().rearrange("(pr p) co -> p pr co", p=128))
            wp8_t = cpool.tile([128, 2, 2, C], FP8, name="wp8_t", tag="wp8_t")
            nc.sync.dma_start(wp8_t[:],
                              wp8.ap().rearrange("(pp s p) co -> p pp s co", p=128, s=2))

            # activation bias (-SHIFT) for the fp8 exp path
            bias_t = cpool.tile([128, 1], F32, name="bias_t", tag="bias_t")
            nc.vector.memset(bias_t[:], -SHIFT)

            # v tiles: [t-chunk, head, 64 v dims | ones | pad]
            v_t = cpool.tile([128, 4, HEADS_PER_CORE, 66], BF, name="v_t", tag="v_t")
            nc.vector.memset(v_t[:, :, :, 64:65], 1.0)
            v8_t = cpool.tile([128, TC, HEADS_PER_CORE, 66], FP8, name="v8_t",
                              tag="v8_t")
            nc.vector.memset(v8_t[:, :, :, 64:65], 1.0)

            # qT/kT per (pair, group); merged yT2 (+ fp8 copy for late proj)
            q_t = [[cpool.tile([128, 512], BF, name=f"q_{p}_{g}", tag=f"q_{p}_{g}")
                    for g in range(TG)] for p in range(PAIRS)]
            k_t = [[cpool.tile([128, 512], BF, name=f"k_{p}_{g}", tag=f"k_{p}_{g}")
                    for g in range(TG)] for p in range(PAIRS)]
            yT2_t = cpool.tile([128, PAIRS, T], BF, name="yT2", tag="yT2")
            yT28_t = cpool.tile([128, PAIRS, TL], FP8, name="yT28", tag="yT28")

            # ---------------- work-item generators ----------------
            def emit_v_chunk0_head01():
                # v for chunk 0, heads 0-1 only: unblocks (pair0, g0)'s av
                # behind just 0.25MB of wv DMA instead of the full 1MB
                psV = ps_util.tile([128, 128], F32, name="psV_0a", tag="util")
                for ci in range(CT):
                    nc.tensor.matmul(
                        psV[:],
                        xT_t[:, ci, 0:128],
                        wv_t[:, ci, 0:128],
                        start=(ci == 0), stop=(ci == CT - 1),
                    )
                pv = psV[:].rearrange("p (h d) -> p h d", d=64)
                nc.vector.tensor_copy(v_t[:, 0, 0:2, 0:64], pv)
                nc.vector.tensor_copy(v8_t[:, 0, 0:2, 0:64], pv)

            def emit_v_chunk0_rest():
                psV = ps_util.tile([128, 384], F32, name="psV_0b", tag="util")
                for ci in range(CT):
                    nc.tensor.matmul(
                        psV[:],
                        xT_t[:, ci, 0:128],
                        wv_t[:, ci, 128:512],
                        start=(ci == 0), stop=(ci == CT - 1),
                    )
                pv = psV[:].rearrange("p (h d) -> p h d", d=64)
                nc.vector.tensor_copy(v_t[:, 0, 2:8, 0:64], pv)
                nc.vector.tensor_copy(v8_t[:, 0, 2:8, 0:64], pv)

            def emit_v_chunk(tc16):
                psV = ps_util.tile([128, 512], F32, name=f"psV_{tc16}", tag="util")
                if tc16 < 4:
                    for ci in range(CT):
                        nc.tensor.matmul(
                            psV[:],
                            xT_t[:, ci, tc16 * 128:(tc16 + 1) * 128],
                            wv_t[:, ci, :],
                            start=(ci == 0), stop=(ci == CT - 1),
                        )
                    nc.vector.tensor_copy(v_t[:, tc16, :, 0:64], psV[:])
                    nc.vector.tensor_copy(v8_t[:, tc16, :, 0:64], psV[:])
                else:
                    tl = (tc16 - 4) * 128
                    for cp in range(CIP):
                        nc.tensor.matmul(
                            psV[:],
                            x8_t[:, 2 * cp:2 * cp + 2, tl:tl + 128],
                            w8v_t[:, cp, :, :],
                            start=(cp == 0), stop=(cp == CIP - 1),
                            perf_mode=DR,
                        )
                    nc.vector.tensor_scalar_mul(v8_t[:, tc16, :, 0:64], psV[:],
                                                1.0 / WS)

            def emit_qk_group(p, g, which):
                ps = ps_util.tile([128, 512], F32, name=f"ps{which}_{p}_{g}",
                                  tag="util")
                if g == 0:
                    w_t = wq_t if which == "q" else wk_t
                    for ci in range(CT):
                        nc.tensor.matmul(
                            ps[:],
                            w_t[:, ci, p * 128:(p + 1) * 128],
                            xT_t[:, ci, :],
                            start=(ci == 0), stop=(ci == CT - 1),
                        )
                    if which == "q":
                        nc.vector.tensor_copy(q_t[p][0][:], ps[:])
                    else:
                        nc.vector.tensor_scalar_mul(k_t[p][0][:], ps[:], LOG2E)
                else:
                    w8_t = w8q_t if which == "q" else w8k_t
                    tl = (g - 1) * 512
                    for cp in range(CIP):
                        nc.tensor.matmul(
                            ps[:],
                            w8_t[:, cp, :, p * 128:(p + 1) * 128],
                            x8_t[:, 2 * cp:2 * cp + 2, tl:tl + 512],
                            start=(cp == 0), stop=(cp == CIP - 1),
                            perf_mode=DR,
                        )
                    # the psQ->q_t copy can otherwise queue ~2.3us behind
                    # z-muls in DVE's in-order queue at group transitions,
                    # delaying the group's first QK^T; ACT idles right there.
                    if which == "q":
                        if _QK_COPY_ACT >= 2:
                            nc.scalar.activation(q_t[p][g][:], ps[:], AF.Copy,
                                                 scale=1.0 / WS)
                        else:
                            nc.vector.tensor_scalar_mul(q_t[p][g][:], ps[:],
                                                        1.0 / WS)
                    else:
                        if _QK_COPY_ACT >= 1:
                            nc.scalar.activation(k_t[p][g][:], ps[:], AF.Copy,
                                                 scale=LOG2E / WS)
                        else:
                            nc.vector.tensor_scalar_mul(k_t[p][g][:], ps[:],
                                                        LOG2E / WS)

            # ---------------- projection chunk ----------------
            proj_ps = {}

            def _proj_mm_early(psZ, p, tc16, co2, start, stop):
                nc.tensor.matmul(
                    psZ[:],
                    yT2_t[:, p, tc16 * 128:(tc16 + 1) * 128],
                    wp_t[:, p, co2 * 512:(co2 + 1) * 512],
                    start=start, stop=stop,
                )

            def _proj_mm_late(psZ, pp, tc16, co2, start, stop):
                tl = (tc16 - 4) * 128
                nc.tensor.matmul(
                    psZ[:],
                    yT28_t[:, 2 * pp:2 * pp + 2, tl:tl + 128],
                    wp8_t[:, pp, :, co2 * 512:(co2 + 1) * 512],
                    start=start, stop=stop,
                    perf_mode=DR,
                )

            def emit_proj_head(tc16, parts):
                # start the psZ accumulation with a subset of contraction
                # parts (pairs for early chunks, pair-pairs for late)
                tiles = []
                for co2 in range(2):
                    psZ = ps_util.tile([128, 512], F32, name=f"psZ_{tc16}_{co2}",
                                       tag="util")
                    for i, pt in enumerate(parts):
                        if tc16 < 4:
                            _proj_mm_early(psZ, pt, tc16, co2, i == 0, False)
                        else:
                            _proj_mm_late(psZ, pt, tc16, co2, i == 0, False)
                    tiles.append(psZ)
                proj_ps[tc16] = (tiles, parts[-1] + 1)

            def emit_proj_tail(tc16):
                tiles, p0 = proj_ps.pop(tc16)
                nparts = PAIRS if tc16 < 4 else 2
                for co2 in range(2):
                    psZ = tiles[co2]
                    for pt in range(p0, nparts):
                        if tc16 < 4:
                            _proj_mm_early(psZ, pt, tc16, co2, False,
                                           pt == nparts - 1)
                        else:
                            _proj_mm_late(psZ, pt, tc16, co2, False,
                                          pt == nparts - 1)
                    z = spool.tile([128, 512], F32, name="z", tag="z", bufs=6)
                    if tc16 < 4:
                        nc.vector.tensor_copy(z[:], psZ[:])
                    elif tc16 >= 13:
                        nc.scalar.activation(z[:], psZ[:], AF.Copy, scale=ZINV)
                    else:
                        nc.vector.tensor_scalar_mul(z[:], psZ[:], ZINV)
                    eng = nc.gpsimd if (co2 and tc16 < 13) else nc.sync
                    eng.dma_start(
                        y.ap()[tc16 * 128:(tc16 + 1) * 128, co2 * 512:(co2 + 1) * 512],
                        z[:],
                    )

            def emit_proj_chunk(tc16):
                if tc16 < 4:
                    emit_proj_head(tc16, list(range(PAIRS - 1)))
                else:
                    emit_proj_head(tc16, [0])
                emit_proj_tail(tc16)

            def emit_proj_co2(tc16, co2):
                # self-contained half-chunk (one psZ): popped as separate
                # work items so a stalled psZ slot can't wedge the PE queue
                # in front of the next attention chunk's QK^T
                psZ = ps_util.tile([128, 512], F32, name=f"psZc_{tc16}_{co2}",
                                   tag="util")
                nparts = PAIRS if tc16 < 4 else 2
                for pt in range(nparts):
                    if tc16 < 4:
                        _proj_mm_early(psZ, pt, tc16, co2, pt == 0,
                                       pt == nparts - 1)
                    else:
                        _proj_mm_late(psZ, pt, tc16, co2, pt == 0,
                                      pt == nparts - 1)
                z = spool.tile([128, 512], F32, name="z", tag="z", bufs=6)
                if tc16 < 4:
                    nc.vector.tensor_copy(z[:], psZ[:])
                elif tc16 >= 13:
                    nc.scalar.activation(z[:], psZ[:], AF.Copy, scale=ZINV)
                else:
                    nc.vector.tensor_scalar_mul(z[:], psZ[:], ZINV)
                eng = nc.gpsimd if (co2 and tc16 < 13) else nc.sync
                eng.dma_start(
                    y.ap()[tc16 * 128:(tc16 + 1) * 128,
                           co2 * 512:(co2 + 1) * 512],
                    z[:],
                )

            # ---------------- deadline-ordered work queue ----------------
            work = []   # [deadline_row, pe_bearing, fn, done, is_qk]

            def add_work(row, pe, fn, front=False, qk=False):
                it = [row, pe, fn, False, qk]
                if front:
                    work.insert(0, it)
                else:
                    work.append(it)

            def run_item(it):
                it[3] = True
                it[2]()

            def pop_work(row, normal):
                got_pe = False
                for it in work:
                    if it[3] or got_pe:
                        continue
                    if it[0] <= row or (normal and it[0] <= row + 2):
                        run_item(it)
                        got_pe = it[1]
                    elif not normal:
                        break
                while work and work[0][3]:
                    work.pop(0)

            def flush_work(row):
                for it in work:
                    if not it[3] and (it[0] < row or (it[0] == row and it[4])):
                        run_item(it)
                while work and work[0][3]:
                    work.pop(0)

            # ---------------- exp helpers ----------------
            def emit_exp(p, g, c, dest_ap_bf, dest_ap_fp8, psS, jofs):
                """exp of one chunk; g0 -> bf16 on ACT, late -> fp8 on
                ACT (exp) or DVE (schraudolph) chosen per chunk."""
                if g == 0:
                    nc.scalar.activation(dest_ap_bf, psS[:, :, jofs:512],
                                         AF.Exp, scale=KAPPA)
                    return
                use_dve = (c + 4 * g + p) % _SCHR_MOD == _SCHR_MOD - 1 \
                    if _EXP_MODE == "mix" else (_EXP_MODE == "schr")
                if use_dve:
                    nc.vector.tensor_scalar(
                        dest_ap_fp8.bitcast(U8), psS[:, :, jofs:512],
                        S0, 119.0,
                        op0=mybir.AluOpType.add, op1=mybir.AluOpType.min,
                    )
                else:
                    nc.scalar.activation(dest_ap_fp8, psS[:, :, jofs:512],
                                         AF.Exp, scale=KAPPA, bias=bias_t[:, 0:1])

            # ---------------- attention: g == 0 (bf16 path) ----------------
            def emit_attention_g0(p):
                g, row, slot = 0, 4 * p, 4 * p
                psY = ps_y.tile([128, 2, 4, 64], F32, name=f"psY_{p}_0", tag="y")
                started = False
                pending_av = None
                for c in range(4):
                    j0 = c
                    jofs = 128 * j0
                    psS = ps_s.tile([128, 2, 512], F32, name=f"psS_{p}_0_{c}",
                                    tag="s")
                    for h in range(2):
                        nc.tensor.matmul(
                            psS[:, h, jofs:512],
                            k_t[p][0][h * 64:(h + 1) * 64, c * 128:(c + 1) * 128],
                            q_t[p][0][h * 64:(h + 1) * 64, jofs:512],
                            start=True, stop=True,
                        )
                    pT = spool.tile([128, 2, 512], BF, name="pT", tag="pT", bufs=4)
                    emit_exp(p, 0, c, pT[:, :, jofs:512], None, psS, jofs)
                    nc.vector.tensor_mul(
                        pT[:, :, jofs:jofs + 128],
                        pT[:, :, jofs:jofs + 128],
                        tri_t[:, None, :].to_broadcast([128, 2, 128]),
                    )
                    pop_work(row, normal=(c % 2 == 1))

                    def emit_av(c=c, j0=j0, pT=pT):
                        nonlocal started
                        order = list(range(j0 + 1, 4)) + [j0]
                        for h in range(2):
                            for j in order:
                                nc.tensor.matmul(
                                    psY[:, h, j, :],
                                    pT[:, h, 128 * j:128 * (j + 1)],
                                    v_t[:, c, 2 * p + h, 0:64],
                                    start=not started,
                                    stop=(c == 3 and h == 1),
                                    skip_group_check=True,
                                )
                                started = True
                                nc.tensor.matmul(
                                    psDen[:, slot, h, j:j + 1],
                                    pT[:, h, 128 * j:128 * (j + 1)],
                                    v_t[:, c, 2 * p + h, 64:65],
                                    start=False, stop=False,
                                    skip_group_check=True,
                                )

                    # defer av by one chunk (in-order PE queue; see late path)
                    if pending_av is not None:
                        pending_av()
                    pending_av = emit_av
                if pending_av is not None:
                    pending_av()
                return psY

            # ------------- attention: g >= 1 (fp8 DoubleRow path) -------------
            def emit_attention_late(p, g):
                row = 4 * p + g
                slot = 4 * p + g
                npairs = 2 * g + 2
                psY = ps_y.tile([128, 2, 4, 64], F32, name=f"psY_{p}_{g}", tag="y")
                started = False
                pending_av = None
                for m in range(npairs):
                    c0, c1 = 2 * m, 2 * m + 1
                    pT8 = spool.tile([128, 2, 2, 512], FP8, name="pT8", tag="pT8",
                                     bufs=6)
                    for c in (c0, c1):
                        diag = c >= 4 * g
                        j0 = (c - 4 * g) if diag else 0
                        jofs = 128 * j0
                        psS = ps_s.tile([128, 2, 512], F32,
                                        name=f"psS_{p}_{g}_{c}", tag="s")
                        kg, kc = c // 4, c % 4
                        for h in range(2):
                            nc.tensor.matmul(
                                psS[:, h, jofs:512],
                                k_t[p][kg][h * 64:(h + 1) * 64,
                                           kc * 128:(kc + 1) * 128],
                                q_t[p][g][h * 64:(h + 1) * 64, jofs:512],
                                start=True, stop=True,
                            )
                        emit_exp(p, g, c, None, pT8[:, c % 2, :, jofs:512],
                                 psS, jofs)
                        if diag:
                            nc.vector.tensor_mul(
                                pT8[:, c % 2, :, jofs:jofs + 128],
                                pT8[:, c % 2, :, jofs:jofs + 128],
                                tri_t[:, None, :].to_broadcast([128, 2, 128]),
                            )
                        pop_work(row, normal=(c % 2 == 1))

                    # att@V for the pair: DR for blocks valid in both chunks,
                    # solo fp8 for the straddle block (c0 diagonal).  Emission
                    # is DEFERRED by one pair: PE queues are in-order, so an
                    # av emitted right after its own QKs blocks the next
                    # pair's QKs behind the exp wait.
                    def emit_av(m=m, c0=c0, c1=c1, pT8=pT8):
                        nonlocal started
                        j0_solo = (c0 - 4 * g) if c0 >= 4 * g else None
                        j_dr0 = max(0, c1 - 4 * g)
                        last_pair = (m == npairs - 1)
                        for h in range(2):
                            if j0_solo is not None:
                                nc.tensor.matmul(
                                    psY[:, h, j0_solo, :],
                                    pT8[:, 0, h,
                                        128 * j0_solo:128 * (j0_solo + 1)],
                                    v8_t[:, c0, 2 * p + h, 0:64],
                                    start=not started, stop=False,
                                    skip_group_check=True,
                                )
                                started = True
                                nc.tensor.matmul(
                                    psDen[:, slot, h, j0_solo:j0_solo + 1],
                                    pT8[:, 0, h,
                                        128 * j0_solo:128 * (j0_solo + 1)],
                                    v8_t[:, c0, 2 * p + h, 64:65],
                                    start=False, stop=False,
                                    skip_group_check=True,
                                )
                            for j in range(j_dr0, 4):
                                nc.tensor.matmul(
                                    psY[:, h, j, :],
                                    pT8[:, :, h, 128 * j:128 * (j + 1)],
                                    v8_t[:, c0:c0 + 2, 2 * p + h, 0:64],
                                    start=not started,
                                    stop=(last_pair and h == 1 and j == 3),
                                    perf_mode=DR,
                                    skip_group_check=True,
                                )
                                started = True
                                nc.tensor.matmul(
                                    psDen[:, slot, h, j:j + 1],
                                    pT8[:, :, h, 128 * j:128 * (j + 1)],
                                    v8_t[:, c0:c0 + 2, 2 * p + h, 64:65],
                                    start=False, stop=False,
                                    perf_mode=DR,
                                    skip_group_check=True,
                                )

                    if pending_av is not None:
                        pending_av()
                    pending_av = emit_av
                if pending_av is not None:
                    pending_av()
                return psY

            # ---- attention: last (pair, group) blockwise (fp8 solo path) ----
            def finalize_block(p, g, j, psY):
                # normalize straight out of PSUM: psY persists for the whole
                # blockwise group anyway, so no staging copy is needed and the
                # serial finalize->transpose chain is one DVE hop shorter
                slot = 4 * p + g
                recB = spool.tile([128, 2, 1], F32, name="recB", tag="recB",
                                  bufs=2)
                nc.vector.reciprocal(recB[:], psDen[:, slot, :, j:j + 1])
                y2B = spool.tile([128, 2, 64], BF, name="y2B", tag="y2B", bufs=2)
                nc.vector.tensor_mul(
                    y2B[:], psY[:, :, j, :],
                    recB[:, :, 0:1].to_broadcast([128, 2, 64]))
                t0 = g * 512 + j * 128
                nc.scalar.dma_start_transpose(yT2_t[:, p, t0:t0 + 128], y2B[:])
                if g >= 1:
                    nc.gpsimd.tensor_scalar_mul(
                        yT28_t[:, p, t0 - 512:t0 - 384],
                        yT2_t[:, p, t0:t0 + 128], YS)

            def emit_attention_blockwise(p, g):
                row = 4 * p + g
                slot = 4 * p + g
                nchunks = 4 * g + 4
                psY = ps_y.tile([128, 2, 4, 64], F32, name=f"psY_{p}_{g}", tag="y")
                started = False
                for c in range(nchunks):
                    diag = c >= 4 * g
                    j0 = (c - 4 * g) if diag else 0
                    jofs = 128 * j0
                    psS = ps_s.tile([128, 2, 512], F32, name=f"psS_{p}_{g}_{c}",
                                    tag="s")
                    kg, kc = c // 4, c % 4
                    for h in range(2):
                        nc.tensor.matmul(
                            psS[:, h, jofs:512],
                            k_t[p][kg][h * 64:(h + 1) * 64,
                                       kc * 128:(kc + 1) * 128],
                            q_t[p][g][h * 64:(h + 1) * 64, jofs:512],
                            start=True, stop=True,
                        )
                    pT8 = spool.tile([128, 2, 512], FP8, name="pT8b", tag="pT8b",
                                     bufs=6)
                    emit_exp(p, g, c, None, pT8[:, :, jofs:512], psS, jofs)
                    if diag:
                        nc.vector.tensor_mul(
                            pT8[:, :, jofs:jofs + 128],
                            pT8[:, :, jofs:jofs + 128],
                            tri_t[:, None, :].to_broadcast([128, 2, 128]),
                        )
                    pop_work(row, normal=(c % 2 == 1))
                    order = list(range(j0 + 1, 4)) + [j0] if diag else range(4)
                    for h in range(2):
                        for j in order:
                            stop = (c == 4 * g + j)
                            nc.tensor.matmul(
                                psY[:, h, j, :],
                                pT8[:, h, 128 * j:128 * (j + 1)],
                                v8_t[:, c, 2 * p + h, 0:64],
                                start=not started, stop=stop,
                                skip_group_check=True,
                            )
                            started = True
                            nc.tensor.matmul(
                                psDen[:, slot, h, j:j + 1],
                                pT8[:, h, 128 * j:128 * (j + 1)],
                                v8_t[:, c, 2 * p + h, 64:65],
                                start=False, stop=False,
                                skip_group_check=True,
                            )
                    if diag:
                        finalize_block(p, g, j0, psY)
                        if 1 <= j0 < 3:
                            emit_proj_chunk(4 * g + j0 - 1)
                        elif j0 == 3:
                            flush_work(1000)
                            emit_proj_chunk(4 * g + 2)
                            emit_proj_head(4 * g + 3, [0])

            # g0 variant of the blockwise tail: bf16 path, per-block finalize
            # feeding the (cheap, transpose-only) early projection chunks.
            # Run LAST so the expensive g3 projections overlap its attention.
            def emit_attention_g0_blockwise(p, row):
                g, slot = 0, 4 * p
                psY = ps_y.tile([128, 2, 4, 64], F32, name=f"psY_{p}_0b",
                                tag="y")
                started = False
                for c in range(4):
                    j0 = c
                    jofs = 128 * j0
                    psS = ps_s.tile([128, 2, 512], F32, name=f"psS_{p}_0b_{c}",
                                    tag="s")
                    for h in range(2):
                        nc.tensor.matmul(
                            psS[:, h, jofs:512],
                            k_t[p][0][h * 64:(h + 1) * 64, c * 128:(c + 1) * 128],
                            q_t[p][0][h * 64:(h + 1) * 64, jofs:512],
                            start=True, stop=True,
                        )
                    pT = spool.tile([128, 2, 512], BF, name="pT", tag="pT", bufs=4)
                    emit_exp(p, 0, c, pT[:, :, jofs:512], None, psS, jofs)
                    nc.vector.tensor_mul(
                        pT[:, :, jofs:jofs + 128],
                        pT[:, :, jofs:jofs + 128],
                        tri_t[:, None, :].to_broadcast([128, 2, 128]),
                    )
                    pop_work(row, normal=(c % 2 == 1))
                    order = list(range(j0 + 1, 4)) + [j0]
                    for h in range(2):
                        for j in order:
                            nc.tensor.matmul(
                                psY[:, h, j, :],
                                pT[:, h, 128 * j:128 * (j + 1)],
                                v_t[:, c, 2 * p + h, 0:64],
                                start=not started, stop=(c == j),
                                skip_group_check=True,
                            )
                            started = True
                            nc.tensor.matmul(
                                psDen[:, slot, h, j:j + 1],
                                pT[:, h, 128 * j:128 * (j + 1)],
                                v_t[:, c, 2 * p + h, 64:65],
                                start=False, stop=False,
                                skip_group_check=True,
                            )
                    finalize_block(p, 0, j0, psY)
                    if 1 <= j0 < 3:
                        emit_proj_chunk(j0 - 1)
                    elif j0 == 3:
                        flush_work(1000)
                        emit_proj_chunk(2)
                        emit_proj_head(3, [0, 1, 2])

            # lazy finalize for non-blockwise groups
            def make_finalize(p, g, psY):
                slot = 4 * p + g
                yraw = spool.tile([128, 2, 4, 64], F32, name="yraw", tag="yraw",
                                  bufs=3)
                nc.vector.tensor_copy(yraw[:], psY[:])

                def finalize(p=p, g=g, yraw=yraw, slot=slot):
                    rec = spool.tile([128, 2, 4], F32, name="rec", tag="rec",
                                     bufs=3)
                    nc.vector.reciprocal(rec[:], psDen[:, slot, :, :])
                    y2 = spool.tile([128, 4, 2, 64], BF, name="y2", tag="y2",
                                    bufs=3)
                    # one fused normalize: y2[p,j,h,d] = yraw[p,h,j,d] *
                    # rec[p,h,j] via stride tricks
                    yraw_b = yraw[:].rearrange("p h j d -> p j h d")
                    rec_b = rec[:].rearrange("p h j -> p j h")[
                        :, :, :, None].to_broadcast([128, 4, 2, 64])
                    eng = nc.gpsimd if _Y2_POOL else nc.vector
                    eng.tensor_mul(y2[:], yraw_b, rec_b)
                    for j in range(4):
                        t0 = g * 512 + j * 128
                        nc.sync.dma_start_transpose(yT2_t[:, p, t0:t0 + 128],
                                                    y2[:, j])
                    if g >= 1:
                        nc.gpsimd.tensor_scalar_mul(
                            yT28_t[:, p, (g - 1) * 512:g * 512],
                            yT2_t[:, p, g * 512:(g + 1) * 512], YS)

                return finalize

            # ---------------- emission schedule ----------------
            for t in range(1, TC):
                add_work(t // 4, True, lambda t=t: emit_v_chunk(t))
            for p in range(PAIRS):
                for g in range(TG):
                    if (p, g) == (0, 0):
                        continue
                    add_work(4 * p + g, True,
                             lambda p=p, g=g: emit_qk_group(p, g, "k"), qk=True)
                    add_work(4 * p + g, True,
                             lambda p=p, g=g: emit_qk_group(p, g, "q"), qk=True)

            emit_v_chunk(0)
            emit_qk_group(0, 0, "k")
            emit_qk_group(0, 0, "q")

            pending = None
            for p in range(PAIRS):
                for g in range(TG):
                    row = 4 * p + g
                    flush_work(row)
                    if pending is not None:
                        add_work(row, False, pending, front=True)
                        pending = None
                    if p == PAIRS - 1 and g >= 1:
                        for t in range(4 * (g - 1), 4 * g):
                            for co2 in range(2):
                                add_work(row + 2, True,
                                         lambda t=t, co2=co2:
                                         emit_proj_co2(t, co2))
                    if p == PAIRS - 1 and g == TG - 1:
                        emit_attention_blockwise(p, g)
                        pending = None
                    else:
                        psY = emit_attention_g0(p) if g == 0 \
                            else emit_attention_late(p, g)
                        pending = make_finalize(p, g, psY)

            if pending is not None:
                pending()
            flush_work(100)
            emit_proj_tail(TC - 1)

            if _DEBUG:
                dq = nc.declare_dram_parameter("dbg_q", [128, 512], F32,
                                               isOutput=True)
                dk = nc.declare_dram_parameter("dbg_k", [128, 512], F32,
                                               isOutput=True)
                dv = nc.declare_dram_parameter("dbg_v", [128, 8, 65], F32,
                                               isOutput=True)
                dy = nc.declare_dram_parameter("dbg_yT28", [128, 512], F32,
                                               isOutput=True)
                for src, dst in ((q_t[0][1], dq), (k_t[0][1], dk)):
                    tmp = spool.tile([128, 512], F32, name="dbgt", tag="dbgt",
                                     bufs=2)
                    nc.vector.tensor_copy(tmp[:], src[:])
                    nc.sync.dma_start(dst.ap(), tmp[:])
                tmpv = spool.tile([128, 8, 65], F32, name="dbgv", tag="dbgv")
                nc.vector.tensor_copy(tmpv[:], v8_t[:, 5, :, 0:65])
                nc.sync.dma_start(dv.ap(), tmpv[:])
                tmpy = spool.tile([128, 512], F32, name="dbgy", tag="dbgy")
                nc.vector.tensor_copy(tmpy[:], yT28_t[:, 0, 0:512])
                nc.sync.dma_start(dy.ap(), tmpy[:])

    nc.compile()
    return nc


def _get_compiled():
    global _compiled
    if _compiled is None:
        _compiled = _build()
    return _compiled


def kernel(x, W_attn, W_proj, _trace=False):
    x = np.asarray(x)
    W_attn = np.asarray(W_attn)
    W_proj = np.asarray(W_proj)
    nc = _get_compiled()

    BF16 = ml_dtypes.bfloat16
    FP8NP = ml_dtypes.float8_e4m3
    tri_np = np.triu(np.ones((128, 128), np.float32)).astype(BF16)
    in_maps = []
    for core in range(N_CORES):
        b, hg = core // 2, core % 2
        cols = slice(hg * 512, (hg + 1) * 512)
        xTb = np.ascontiguousarray(x[b].T)
        wqs = W_attn[:, 0 * C:1 * C][:, cols]
        wks = W_attn[:, 1 * C:2 * C][:, cols]
        wvs = W_attn[:, 2 * C:3 * C][:, cols]
        wps = W_proj[hg * 512:(hg + 1) * 512, :]
        in_maps.append({
            "xT": xTb[:, :512].astype(BF16),
            "x8": xTb[:, 512:].astype(FP8NP),
            "wq": wqs.astype(BF16),
            "wk": wks.astype(BF16),
            "wv": wvs.astype(BF16),
            "w8q": (wqs * np.float32(WS)).astype(FP8NP),
            "w8k": (wks * np.float32(WS)).astype(FP8NP),
            "w8v": (wvs * np.float32(WS)).astype(FP8NP),
            "wp": wps.astype(BF16),
            "wp8": (wps * np.float32(WS)).astype(FP8NP),
            "tri": tri_np,
        })

    res = run_bass_kernel_spmd(nc, in_maps, list(range(N_CORES)), trace=_trace)
    out = np.empty((B, T, C), np.float32)
    for b in range(B):
        out[b] = res.results[2 * b]["y"] + res.results[2 * b + 1]["y"]
    if _trace:
        kernel._last_exec_time_ns = res.exec_time_ns
        kernel._last_results = res
    return out


# revision 67
# speedup vs baseline: 1.0202x; 1.0202x over previous
"""Causal self-attention kernel for 8 TRN2 NeuronCores.

Problem: B=4, T=2048, C=1024, H=16 heads, D=64 (fp32 in/out).

Sharding: 8 cores = 4 batch entries x 2 head-groups (8 heads each).
Each core computes, for its (batch b, head-group hg):
    qkv slice -> flash-style causal attention (no-max softmax) -> partial
    projection y_part = attn_out @ W_proj[rows of its heads].
Host sums the two partial projections per batch entry.

Precision strategy (row-split): query rows t<512 (tq-group g=0) follow the
bf16 path everywhere (errors there don't average over keys, so they must
stay accurate); rows t>=512 use fp8e4 DoubleRow matmuls (0.5 cyc/row, 2x
effective contraction) for qkv-gen, att@V and the projection, plus fp8
attention weights produced either by ACT exp (fp8 out, shifted by -SHIFT to
fit e4m3's 240 max) or by a one-op DVE "Schraudolph" exp: psS already holds
S*log2e (k_t is pre-scaled by log2e), so uint8(round(min(psS + S0, 119)))
bit-cast to fp8e4 IS exp(s-SHIFT) to ~3%; uint8 saturation-at-0 provides
the lower clamp. The softmax ratio cancels the shift exactly.

fp8 operands are pre-scaled into e4m3's sweet spot (W by 32, y2 by 4) and
the inverse scales are folded into existing DVE copies, so they are free.

On-device layout:
  - QK^T contracts 64 dims/head (bf16, psS = S*log2e); exp -> pT (g0: bf16;
    g>=1: fp8 in chunk-PAIRED tiles [128, 2, 2h, 512]).
  - att@V g0: per-(chunk,head,block) solo bf16 matmuls (64-col moving);
    g>=1: DoubleRow over chunk pairs (stationary [tk,2,128tq] pT-pair,
    moving [tk,2,64] v-pair), deferred by one pair so the in-order PE queue
    never blocks the next pair's QK^T behind the exp wait.
  - psY is exactly ONE PSUM bank per group ([128,2h,4j,64]); all 16 groups'
    softmax denominators accumulate into a single shared PSUM bank via
    start=False 1-col matmuls (bank memset once; start_tensor_calc would
    wipe the whole bank, so it is never used there).
  - projection: chunks 0-3 bf16 (4-pair contraction); chunks 4-15 DoubleRow
    over pair-pairs from yT2_8 (fp8 copy of the transposed normalized y2).
  - diag masking via tri multiply on DVE (bf16 for g0, fp8 late;
    gpsimd masking loses ~4.7us: Pool queues them behind SWDGE desc-gen).
Software-pipelined emission: deadline-ordered work queue of qkv/proj chunks
interleaved into the ACT/DVE-paced attention loop, as before.
PSUM budget: 2 util + 2x2 psS + 1 psY + 1 den = 8 banks.
TimelineSim: 203.5us (baseline bf16 kernel: 241.5us); rel err 1.08e-2.
"""

import numpy as np
import ml_dtypes
import sys

sys.path.insert(0, "/opt/trn_rl_repo")

import concourse.bass as bass
import concourse.mybir as mybir
import concourse.tile as tile
from concourse import bacc
from concourse.bass_utils import run_bass_kernel_spmd

BF = mybir.dt.bfloat16
F32 = mybir.dt.float32
FP8 = mybir.dt.float8e4
U8 = mybir.dt.uint8
AF = mybir.ActivationFunctionType
DR = mybir.MatmulPerfMode.DoubleRow

B, T, C = 4, 2048, 1024
H, D = 16, 64
N_CORES = 8
HEADS_PER_CORE = 8          # 4 pairs
PAIRS = 4
TC = T // 128               # 16 t-chunks of 128
TG = T // 512               # 4 t-groups of 512
CT = C // 128               # 8 contraction tiles
CIP = CT // 2               # 4 contraction-tile pairs (DoubleRow)
TL = T - 512                # late-row span (t >= 512)

LOG2E = 1.4426950408889634
KAPPA = 1.0 / (8.0 * LOG2E)     # activation scale: exp(psS*KAPPA) = exp(s~)
SHIFT = 3.2                     # p~ = exp(s~ - SHIFT) keeps e4m3 finite
DELTA = 0.32                    # schraudolph interp-error centering
S0 = 56.0 - DELTA - SHIFT * 8.0 * LOG2E
WS = 32.0                       # fp8 weight pre-scale
YS = 4.0                        # fp8 y2 pre-scale
ZINV = 1.0 / (WS * YS)          # late proj PSUM descale

_compiled = None
_DEBUG = False
_EXP_MODE = "mix"   # "act" | "schr" | "mix"
_SCHR_MOD = 8       # in mix mode: chunk is schraudolph iff (c+4g+p)%MOD==MOD-1
_Y2_POOL = False    # fused y2 normalize on gpsimd instead of DVE
_QK_COPY_ACT = 0    # 0=DVE; 1=late k copies on ACT; 2=late q+k copies on ACT


def _build():
    nc = bacc.Bacc("TRN2", target_bir_lowering=False)

    xT = nc.declare_dram_parameter("xT", [C, 512], BF, isOutput=False)
    x8 = nc.declare_dram_parameter("x8", [C, TL], FP8, isOutput=False)
    wq = nc.declare_dram_parameter("wq", [C, 512], BF, isOutput=False)
    wk = nc.declare_dram_parameter("wk", [C, 512], BF, isOutput=False)
    wv = nc.declare_dram_parameter("wv", [C, 512], BF, isOutput=False)
    w8q = nc.declare_dram_parameter("w8q", [C, 512], FP8, isOutput=False)
    w8k = nc.declare_dram_parameter("w8k", [C, 512], FP8, isOutput=False)
    w8v = nc.declare_dram_parameter("w8v", [C, 512], FP8, isOutput=False)
    wp = nc.declare_dram_parameter("wp", [512, C], BF, isOutput=False)
    wp8 = nc.declare_dram_parameter("wp8", [512, C], FP8, isOutput=False)
    tri = nc.declare_dram_parameter("tri", [128, 128], BF, isOutput=False)
    y = nc.declare_dram_parameter("y", [T, C], F32, isOutput=True)

    with tile.TileContext(nc) as tc:
        with (
            tc.tile_pool(name="const", bufs=1) as cpool,
            tc.tile_pool(name="small", bufs=2) as spool,
            tc.tile_pool(name="ps_util", bufs=2, space="PSUM") as ps_util,
            tc.tile_pool(name="ps_s", bufs=2, space="PSUM") as ps_s,
            tc.tile_pool(name="ps_y", bufs=1, space="PSUM") as ps_y,
            tc.tile_pool(name="ps_den", bufs=1, space="PSUM") as ps_den,
        ):
            # softmax denominators for all 16 (pair, group)s live in one
            # shared PSUM bank: zeroed once, then accumulated into by
            # start=False 1-col matmuls (never start_tensor_calc'd, so no
            # bank wipes).  This keeps each psY to exactly one bank.
            psDen = ps_den.tile([128, 16, 2, 4], F32, name="psDen", tag="den")
            nc.vector.memset(psDen[:], 0.0)
            # ---------------- constant loads ----------------
            xT_t = cpool.tile([128, CT, 512], BF, name="xT_t", tag="xT_t")
            x8_t = cpool.tile([128, CT, TL], FP8, name="x8_t", tag="x8_t")
            wv_t = cpool.tile([128, CT, 512], BF, name="wv_t", tag="wv_t")
            wq_t = cpool.tile([128, CT, 512], BF, name="wq_t", tag="wq_t")
            wk_t = cpool.tile([128, CT, 512], BF, name="wk_t", tag="wk_t")
            w8v_t = cpool.tile([128, CIP, 2, 512], FP8, name="w8v_t", tag="w8v_t")
            w8q_t = cpool.tile([128, CIP, 2, 512], FP8, name="w8q_t", tag="w8q_t")
            w8k_t = cpool.tile([128, CIP, 2, 512], FP8, name="w8k_t", tag="w8k_t")
            tri_t = cpool.tile([128, 128], BF, name="tri_t", tag="tri_t")

            xT_r = xT.ap().rearrange("(ct p) t -> p ct t", p=128)
            nc.sync.dma_start(wv_t[:], wv.ap().rearrange("(ct p) d -> p ct d", p=128))
            nc.sync.dma_start(xT_t[:, :, 0:128], xT_r[:, :, 0:128])
            nc.sync.dma_start(xT_t[:, :, 128:512], xT_r[:, :, 128:512])
            nc.sync.dma_start(wk_t[:], wk.ap().rearrange("(ct p) d -> p ct d", p=128))
            nc.sync.dma_start(wq_t[:], wq.ap().rearrange("(ct p) d -> p ct d", p=128))
            nc.sync.dma_start(tri_t[:], tri.ap())
            nc.sync.dma_start(w8v_t[:],
                              w8v.ap().rearrange("(cp s p) d -> p cp s d", p=128, s=2))
            nc.sync.dma_start(x8_t[:], x8.ap().rearrange("(ct p) t -> p ct t", p=128))
            nc.sync.dma_start(w8k_t[:],
                              w8k.ap().rearrange("(cp s p) d -> p cp s d", p=128, s=2))
            nc.sync.dma_start(w8q_t[:],
                              w8q.ap().rearrange("(cp s p) d -> p cp s d", p=128, s=2))
            wp_t = cpool.tile([128, PAIRS, C], BF, name="wp_t", tag="wp_t")
            nc.sync.dma_start(wp_t[:], wp.ap().rearrange("(pr p) co -> p pr co", p=128))
            wp8_t = cpool.tile([128, 2, 2, C], FP8, name="wp8_t", tag="wp8_t")
            nc.sync.dma_start(wp8_t[:],
                              wp8.ap().rearrange("(pp s p) co -> p pp s co", p=128, s=2))

            # activation bias (-SHIFT) for the fp8 exp path
            bias_t = cpool.tile([128, 1], F32, name="bias_t", tag="bias_t")
            nc.vector.memset(bias_t[:], -SHIFT)

            # v tiles: [t-chunk, head, 64 v dims | ones | pad]
            v_t = cpool.tile([128, 4, HEADS_PER_CORE, 66], BF, name="v_t", tag="v_t")
            nc.vector.memset(v_t[:, :, :, 64:65], 1.0)
            v8_t = cpool.tile([128, TC, HEADS_PER_CORE, 66], FP8, name="v8_t",
                              tag="v8_t")
            nc.vector.memset(v8_t[:, :, :, 64:65], 1.0)

            # qT/kT per (pair, group); merged yT2 (+ fp8 copy for late proj)
            q_t = [[cpool.tile([128, 512], BF, name=f"q_{p}_{g}", tag=f"q_{p}_{g}")
                    for g in range(TG)] for p in range(PAIRS)]
            k_t = [[cpool.tile([128, 512], BF, name=f"k_{p}_{g}", tag=f"k_{p}_{g}")
                    for g in range(TG)] for p in range(PAIRS)]
            yT2_t = cpool.tile([128, PAIRS, T], BF, name="yT2", tag="yT2")
            yT28_t = cpool.tile([128, PAIRS, TL], FP8, name="yT28", tag="yT28")

            # ---------------- work-item generators ----------------
            def emit_v_chunk0_head01():
                # v for chunk 0, heads 0-1 only: unblocks (pair0, g0)'s av
                # behind just 0.25MB of wv DMA instead of the full 1MB
                psV = ps_util.tile([128, 128], F32, name="psV_0a", tag="util")
                for ci in range(CT):
                    nc.tensor.matmul(
                        psV[:],
                        xT_t[:, ci, 0:128],
                        wv_t[:, ci, 0:128],
                        start=(ci == 0), stop=(ci == CT - 1),
                    )
                pv = psV[:].rearrange("p (h d) -> p h d", d=64)
                nc.vector.tensor_copy(v_t[:, 0, 0:2, 0:64], pv)
                nc.vector.tensor_copy(v8_t[:, 0, 0:2, 0:64], pv)

            def emit_v_chunk0_rest():
                psV = ps_util.tile([128, 384], F32, name="psV_0b", tag="util")
                for ci in range(CT):
                    nc.tensor.matmul(
                        psV[:],
                        xT_t[:, ci, 0:128],
                        wv_t[:, ci, 128:512],
                        start=(ci == 0), stop=(ci == CT - 1),
                    )
                pv = psV[:].rearrange("p (h d) -> p h d", d=64)
                nc.vector.tensor_copy(v_t[:, 0, 2:8, 0:64], pv)
                nc.vector.tensor_copy(v8_t[:, 0, 2:8, 0:64], pv)

            def emit_v_chunk(tc16):
                psV = ps_util.tile([128, 512], F32, name=f"psV_{tc16}", tag="util")
                if tc16 < 4:
                    for ci in range(CT):
                        nc.tensor.matmul(
                            psV[:],
                            xT_t[:, ci, tc16 * 128:(tc16 + 1) * 128],
                            wv_t[:, ci, :],
                            start=(ci == 0), stop=(ci == CT - 1),
                        )
                    nc.vector.tensor_copy(v_t[:, tc16, :, 0:64], psV[:])
                    nc.vector.tensor_copy(v8_t[:, tc16, :, 0:64], psV[:])
                else:
                    tl = (tc16 - 4) * 128
                    for cp in range(CIP):
                        nc.tensor.matmul(
                            psV[:],
                            x8_t[:, 2 * cp:2 * cp + 2, tl:tl + 128],
                            w8v_t[:, cp, :, :],
                            start=(cp == 0), stop=(cp == CIP - 1),
                            perf_mode=DR,
                        )
                    nc.vector.tensor_scalar_mul(v8_t[:, tc16, :, 0:64], psV[:],
                                                1.0 / WS)

            def emit_qk_group(p, g, which):
                ps = ps_util.tile([128, 512], F32, name=f"ps{which}_{p}_{g}",
                                  tag="util")
                if g == 0:
                    w_t = wq_t if which == "q" else wk_t
                    for ci in range(CT):
                        nc.tensor.matmul(
                            ps[:],
                            w_t[:, ci, p * 128:(p + 1) * 128],
                            xT_t[:, ci, :],
                            start=(ci == 0), stop=(ci == CT - 1),
                        )
                    if which == "q":
                        nc.vector.tensor_copy(q_t[p][0][:], ps[:])
                    else:
                        nc.vector.tensor_scalar_mul(k_t[p][0][:], ps[:], LOG2E)
                else:
                    w8_t = w8q_t if which == "q" else w8k_t
                    tl = (g - 1) * 512
                    for cp in range(CIP):
                        nc.tensor.matmul(
                            ps[:],
                            w8_t[:, cp, :, p * 128:(p + 1) * 128],
                            x8_t[:, 2 * cp:2 * cp + 2, tl:tl + 512],
                            start=(cp == 0), stop=(cp == CIP - 1),
                            perf_mode=DR,
                        )
                    # the psQ->q_t copy can otherwise queue ~2.3us behind
                    # z-muls in DVE's in-order queue at group transitions,
                    # delaying the group's first QK^T; ACT idles right there.
                    if which == "q":
                        if _QK_COPY_ACT >= 2:
                            nc.scalar.activation(q_t[p][g][:], ps[:], AF.Copy,
                                                 scale=1.0 / WS)
                        else:
                            nc.vector.tensor_scalar_mul(q_t[p][g][:], ps[:],
                                                        1.0 / WS)
                    else:
                        if _QK_COPY_ACT >= 1:
                            nc.scalar.activation(k_t[p][g][:], ps[:], AF.Copy,
                                                 scale=LOG2E / WS)
                        else:
                            nc.vector.tensor_scalar_mul(k_t[p][g][:], ps[:],
                                                        LOG2E / WS)

            # ---------------- projection chunk ----------------
            proj_ps = {}

            def _proj_mm_early(psZ, p, tc16, co2, start, stop):
                nc.tensor.matmul(
                    psZ[:],
                    yT2_t[:, p, tc16 * 128:(tc16 + 1) * 128],
                    wp_t[:, p, co2 * 512:(co2 + 1) * 512],
                    start=start, stop=stop,
                )

            def _proj_mm_late(psZ, pp, tc16, co2, start, stop):
                tl = (tc16 - 4) * 128
                nc.tensor.matmul(
                    psZ[:],
                    yT28_t[:, 2 * pp:2 * pp + 2, tl:tl + 128],
                    wp8_t[:, pp, :, co2 * 512:(co2 + 1) * 512],
                    start=start, stop=stop,
                    perf_mode=DR,
                )

            def emit_proj_head(tc16, parts):
                # start the psZ accumulation with a subset of contraction
                # parts (pairs for early chunks, pair-pairs for late)
                tiles = []
                for co2 in range(2):
                    psZ = ps_util.tile([128, 512], F32, name=f"psZ_{tc16}_{co2}",
                                       tag="util")
                    for i, pt in enumerate(parts):
                        if tc16 < 4:
                            _proj_mm_early(psZ, pt, tc16, co2, i == 0, False)
                        else:
                            _proj_mm_late(psZ, pt, tc16, co2, i == 0, False)
                    tiles.append(psZ)
                proj_ps[tc16] = (tiles, parts[-1] + 1)

            def emit_proj_tail(tc16):
                tiles, p0 = proj_ps.pop(tc16)
                nparts = PAIRS if tc16 < 4 else 2
                for co2 in range(2):
                    psZ = tiles[co2]
                    for pt in range(p0, nparts):
                        if tc16 < 4:
                            _proj_mm_early(psZ, pt, tc16, co2, False,
                                           pt == nparts - 1)
                        else:
                            _proj_mm_late(psZ, pt, tc16, co2, False,
                                          pt == nparts - 1)
                    z = spool.tile([128, 512], F32, name="z", tag="z", bufs=6)
                    if tc16 < 4:
                        nc.vector.tensor_copy(z[:], psZ[:])
                    elif tc16 >= 13:
                        nc.scalar.activation(z[:], psZ[:], AF.Copy, scale=ZINV)
                    else:
                        nc.vector.tensor_scalar_mul(z[:], psZ[:], ZINV)
                    eng = nc.gpsimd if (co2 and tc16 < 13) else nc.sync
                    eng.dma_start(
                        y.ap()[tc16 * 128:(tc16 + 1) * 128, co2 * 512:(co2 + 1) * 512],
                        z[:],
                    )

            def emit_proj_chunk(tc16):
                if tc16 < 4:
                    emit_proj_head(tc16, list(range(PAIRS - 1)))
                else:
                    emit_proj_head(tc16, [0])
                emit_proj_tail(tc16)

            def emit_proj_co2(tc16, co2):
                # self-contained half-chunk (one psZ): popped as separate
                # work items so a stalled psZ slot can't wedge the PE queue
                # in front of the next attention chunk's QK^T
                psZ = ps_util.tile([128, 512], F32, name=f"psZc_{tc16}_{co2}",
                                   tag="util")
                nparts = PAIRS if tc16 < 4 else 2
                for pt in range(nparts):
                    if tc16 < 4:
                        _proj_mm_early(psZ, pt, tc16, co2, pt == 0,
                                       pt == nparts - 1)
                    else:
                        _proj_mm_late(psZ, pt, tc16, co2, pt == 0,
                                      pt == nparts - 1)
                z = spool.tile([128, 512], F32, name="z", tag="z", bufs=6)
                if tc16 < 4:
                    nc.vector.tensor_copy(z[:], psZ[:])
                elif tc16 >= 13:
                    nc.scalar.activation(z[:], psZ[:], AF.Copy, scale=ZINV)
                else:
                    nc.vector.tensor_scalar_mul(z[:], psZ[:], ZINV)
                eng = nc.gpsimd if (co2 and tc16 < 13) else nc.sync
                eng.dma_start(
                    y.ap()[tc16 * 128:(tc16 + 1) * 128,
                           co2 * 512:(co2 + 1) * 512],
                    z[:],
                )

            # ---------------- deadline-ordered work queue ----------------
            work = []   # [deadline_row, pe_bearing, fn, done, is_qk]

            def add_work(row, pe, fn, front=False, qk=False):
                it = [row, pe, fn, False, qk]
                if front:
                    work.insert(0, it)
                else:
                    work.append(it)

            def run_item(it):
                it[3] = True
                it[2]()

            def pop_work(row, normal):
                got_pe = False
                for it in work:
                    if it[3] or got_pe:
                        continue
                    if it[0] <= row or (normal and it[0] <= row + 2):
                        run_item(it)
                        got_pe = it[1]
                    elif not normal:
                        break
                while work and work[0][3]:
                    work.pop(0)

            def flush_work(row):
                for it in work:
                    if not it[3] and (it[0] < row or (it[0] == row and it[4])):
                        run_item(it)
                while work and work[0][3]:
                    work.pop(0)

            # ---------------- exp helpers ----------------
            def emit_exp(p, g, c, dest_ap_bf, dest_ap_fp8, psS, jofs):
                """exp of one chunk; g0 -> bf16 on ACT, late -> fp8 on
                ACT (exp) or DVE (schraudolph) chosen per chunk."""
                if g == 0:
                    nc.scalar.activation(dest_ap_bf, psS[:, :, jofs:512],
                                         AF.Exp, scale=KAPPA)
                    return
                use_dve = (c + 4 * g + p) % _SCHR_MOD == _SCHR_MOD - 1 \
                    if _EXP_MODE == "mix" else (_EXP_MODE == "schr")
                if use_dve:
                    nc.vector.tensor_scalar(
                        dest_ap_fp8.bitcast(U8), psS[:, :, jofs:512],
                        S0, 119.0,
                        op0=mybir.AluOpType.add, op1=mybir.AluOpType.min,
                    )
                else:
                    nc.scalar.activation(dest_ap_fp8, psS[:, :, jofs:512],
                                         AF.Exp, scale=KAPPA, bias=bias_t[:, 0:1])

            # ---------------- attention: g == 0 (bf16 path) ----------------
            def emit_attention_g0(p):
                g, row, slot = 0, 4 * p, 4 * p
                psY = ps_y.tile([128, 2, 4, 64], F32, name=f"psY_{p}_0", tag="y")
                started = False
                pending_av = None
                for c in range(4):
                    j0 = c
                    jofs = 128 * j0
                    psS = ps_s.tile([128, 2, 512], F32, name=f"psS_{p}_0_{c}",
                                    tag="s")
                    for h in range(2):
                        nc.tensor.matmul(
                            psS[:, h, jofs:512],
                            k_t[p][0][h * 64:(h + 1) * 64, c * 128:(c + 1) * 128],
                            q_t[p][0][h * 64:(h + 1) * 64, jofs:512],
                            start=True, stop=True,
                        )
                    pT = spool.tile([128, 2, 512], BF, name="pT", tag="pT", bufs=4)
                    emit_exp(p, 0, c, pT[:, :, jofs:512], None, psS, jofs)
                    nc.vector.tensor_mul(
                        pT[:, :, jofs:jofs + 128],
                        pT[:, :, jofs:jofs + 128],
                        tri_t[:, None, :].to_broadcast([128, 2, 128]),
                    )
                    pop_work(row, normal=(c % 2 == 1))

                    def emit_av(c=c, j0=j0, pT=pT):
                        nonlocal started
                        order = list(range(j0 + 1, 4)) + [j0]
                        for h in range(2):
                            for j in order:
                                nc.tensor.matmul(
                                    psY[:, h, j, :],
                                    pT[:, h, 128 * j:128 * (j + 1)],
                                    v_t[:, c, 2 * p + h, 0:64],
                                    start=not started,
                                    stop=(c == 3 and h == 1),
                                    skip_group_check=True,
                                )
                                started = True
                                nc.tensor.matmul(
                                    psDen[:, slot, h, j:j + 1],
                                    pT[:, h, 128 * j:128 * (j + 1)],
                                    v_t[:, c, 2 * p + h, 64:65],
                                    start=False, stop=False,
                                    skip_group_check=True,
                                )

                    # defer av by one chunk (in-order PE queue; see late path)
                    if pending_av is not None:
                        pending_av()
                    pending_av = emit_av
                if pending_av is not None:
                    pending_av()
                return psY

            # ------------- attention: g >= 1 (fp8 DoubleRow path) -------------
            def emit_attention_late(p, g):
                row = 4 * p + g
                slot = 4 * p + g
                npairs = 2 * g + 2
                psY = ps_y.tile([128, 2, 4, 64], F32, name=f"psY_{p}_{g}", tag="y")
                started = False
                pending_av = None
                for m in range(npairs):
                    c0, c1 = 2 * m, 2 * m + 1
                    pT8 = spool.tile([128, 2, 2, 512], FP8, name="pT8", tag="pT8",
                                     bufs=6)
                    for c in (c0, c1):
                        diag = c >= 4 * g
                        j0 = (c - 4 * g) if diag else 0
                        jofs = 128 * j0
                        psS = ps_s.tile([128, 2, 512], F32,
                                        name=f"psS_{p}_{g}_{c}", tag="s")
                        kg, kc = c // 4, c % 4
                        for h in range(2):
                            nc.tensor.matmul(
                                psS[:, h, jofs:512],
                                k_t[p][kg][h * 64:(h + 1) * 64,
                                           kc * 128:(kc + 1) * 128],
                                q_t[p][g][h * 64:(h + 1) * 64, jofs:512],
                                start=True, stop=True,
                            )
                        emit_exp(p, g, c, None, pT8[:, c % 2, :, jofs:512],
                                 psS, jofs)
                        if diag:
                            nc.vector.tensor_mul(
                                pT8[:, c % 2, :, jofs:jofs + 128],
                                pT8[:, c % 2, :, jofs:jofs + 128],
                                tri_t[:, None, :].to_broadcast([128, 2, 128]),
                            )
                        pop_work(row, normal=(c % 2 == 1))

                    # att@V for the pair: DR for blocks valid in both chunks,
                    # solo fp8 for the straddle block (c0 diagonal).  Emission
                    # is DEFERRED by one pair: PE queues are in-order, so an
                    # av emitted right after its own QKs blocks the next
                    # pair's QKs behind the exp wait.
                    def emit_av(m=m, c0=c0, c1=c1, pT8=pT8):
                        nonlocal started
                        j0_solo = (c0 - 4 * g) if c0 >= 4 * g else None
                        j_dr0 = max(0, c1 - 4 * g)
                        last_pair = (m == npairs - 1)
                        for h in range(2):
                            if j0_solo is not None:
                                nc.tensor.matmul(
                                    psY[:, h, j0_solo, :],
                                    pT8[:, 0, h,
                                        128 * j0_solo:128 * (j0_solo + 1)],
                                    v8_t[:, c0, 2 * p + h, 0:64],
                                    start=not started, stop=False,
                                    skip_group_check=True,
                                )
                                started = True
                                nc.tensor.matmul(
                                    psDen[:, slot, h, j0_solo:j0_solo + 1],
                                    pT8[:, 0, h,
                                        128 * j0_solo:128 * (j0_solo + 1)],
                                    v8_t[:, c0, 2 * p + h, 64:65],
                                    start=False, stop=False,
                                    skip_group_check=True,
                                )
                            for j in range(j_dr0, 4):
                                nc.tensor.matmul(
                                    psY[:, h, j, :],
                                    pT8[:, :, h, 128 * j:128 * (j + 1)],
                                    v8_t[:, c0:c0 + 2, 2 * p + h, 0:64],
                                    start=not started,
                                    stop=(last_pair and h == 1 and j == 3),
                                    perf_mode=DR,
                                    skip_group_check=True,
                                )
                                started = True
                                nc.tensor.matmul(
                                    psDen[:, slot, h, j:j + 1],
                                    pT8[:, :, h, 128 * j:128 * (j + 1)],
                                    v8_t[:, c0:c0 + 2, 2 * p + h, 64:65],
                                    start=False, stop=False,
                                    perf_mode=DR,
                                    skip_group_check=True,
                                )

                    if pending_av is not None:
                        pending_av()
                    pending_av = emit_av
                if pending_av is not None:
                    pending_av()
                return psY

            # ---- attention: last (pair, group) blockwise (fp8 solo path) ----
            def finalize_block(p, g, j, psY):
                # normalize straight out of PSUM: psY persists for the whole
                # blockwise group anyway, so no staging copy is needed and the
                # serial finalize->transpose chain is one DVE hop shorter
                slot = 4 * p + g
                recB = spool.tile([128, 2, 1], F32, name="recB", tag="recB",
                                  bufs=4)
                nc.vector.reciprocal(recB[:], psDen[:, slot, :, j:j + 1])
                y2B = spool.tile([128, 2, 64], BF, name="y2B", tag="y2B", bufs=4)
                nc.vector.tensor_mul(
                    y2B[:], psY[:, :, j, :],
                    recB[:, :, 0:1].to_broadcast([128, 2, 64]))
                t0 = g * 512 + j * 128
                nc.scalar.dma_start_transpose(yT2_t[:, p, t0:t0 + 128], y2B[:])
                if g >= 1:
                    nc.gpsimd.tensor_scalar_mul(
                        yT28_t[:, p, t0 - 512:t0 - 384],
                        yT2_t[:, p, t0:t0 + 128], YS)

            def emit_attention_blockwise(p, g):
                row = 4 * p + g
                slot = 4 * p + g
                nchunks = 4 * g + 4
                psY = ps_y.tile([128, 2, 4, 64], F32, name=f"psY_{p}_{g}", tag="y")
                started = False
                for c in range(nchunks):
                    diag = c >= 4 * g
                    j0 = (c - 4 * g) if diag else 0
                    jofs = 128 * j0
                    psS = ps_s.tile([128, 2, 512], F32, name=f"psS_{p}_{g}_{c}",
                                    tag="s")
                    kg, kc = c // 4, c % 4
                    for h in range(2):
                        nc.tensor.matmul(
                            psS[:, h, jofs:512],
                            k_t[p][kg][h * 64:(h + 1) * 64,
                                       kc * 128:(kc + 1) * 128],
                            q_t[p][g][h * 64:(h + 1) * 64, jofs:512],
                            start=True, stop=True,
                        )
                    pT8 = spool.tile([128, 2, 512], FP8, name="pT8b", tag="pT8b",
                                     bufs=6)
                    emit_exp(p, g, c, None, pT8[:, :, jofs:512], psS, jofs)
                    if diag:
                        nc.vector.tensor_mul(
                            pT8[:, :, jofs:jofs + 128],
                            pT8[:, :, jofs:jofs + 128],
                            tri_t[:, None, :].to_broadcast([128, 2, 128]),
                        )
                    pop_work(row, normal=(c % 2 == 1))
                    order = list(range(j0 + 1, 4)) + [j0] if diag else range(4)
                    for h in range(2):
                        for j in order:
                            stop = (c == 4 * g + j)
                            nc.tensor.matmul(
                                psY[:, h, j, :],
                                pT8[:, h, 128 * j:128 * (j + 1)],
                                v8_t[:, c, 2 * p + h, 0:64],
                                start=not started, stop=stop,
                                skip_group_check=True,
                            )
                            started = True
                            nc.tensor.matmul(
                                psDen[:, slot, h, j:j + 1],
                                pT8[:, h, 128 * j:128 * (j + 1)],
                                v8_t[:, c, 2 * p + h, 64:65],
                                start=False, stop=False,
                                skip_group_check=True,
                            )
                    if diag:
                        finalize_block(p, g, j0, psY)
                        if 1 <= j0 < 3:
                            emit_proj_chunk(4 * g + j0 - 1)
                        elif j0 == 3:
                            flush_work(1000)
                            emit_proj_chunk(4 * g + 2)
                            emit_proj_head(4 * g + 3, [0])

            # g0 variant of the blockwise tail: bf16 path, per-block finalize
            # feeding the (cheap, transpose-only) early projection chunks.
            # Run LAST so the expensive g3 projections overlap its attention.
            def emit_attention_g0_blockwise(p, row):
                g, slot = 0, 4 * p
                psY = ps_y.tile([128, 2, 4, 64], F32, name=f"psY_{p}_0b",
                                tag="y")
                started = False
                for c in range(4):
                    j0 = c
                    jofs = 128 * j0
                    psS = ps_s.tile([128, 2, 512], F32, name=f"psS_{p}_0b_{c}",
                                    tag="s")
                    for h in range(2):
                        nc.tensor.matmul(
                            psS[:, h, jofs:512],
                            k_t[p][0][h * 64:(h + 1) * 64, c * 128:(c + 1) * 128],
                            q_t[p][0][h * 64:(h + 1) * 64, jofs:512],
                            start=True, stop=True,
                        )
                    pT = spool.tile([128, 2, 512], BF, name="pT", tag="pT", bufs=4)
                    emit_exp(p, 0, c, pT[:, :, jofs:512], None, psS, jofs)
                    nc.vector.tensor_mul(
                        pT[:, :, jofs:jofs + 128],
                        pT[:, :, jofs:jofs + 128],
                        tri_t[:, None, :].to_broadcast([128, 2, 128]),
                    )
                    pop_work(row, normal=(c % 2 == 1))
                    order = list(range(j0 + 1, 4)) + [j0]
                    for h in range(2):
                        for j in order:
                            nc.tensor.matmul(
                                psY[:, h, j, :],
                                pT[:, h, 128 * j:128 * (j + 1)],
                                v_t[:, c, 2 * p + h, 0:64],
                                start=not started, stop=(c == j),
                                skip_group_check=True,
                            )
                            started = True
                            nc.tensor.matmul(
                                psDen[:, slot, h, j:j + 1],
                                pT[:, h, 128 * j:128 * (j + 1)],
                                v_t[:, c, 2 * p + h, 64:65],
                                start=False, stop=False,
                                skip_group_check=True,
                            )
                    finalize_block(p, 0, j0, psY)
                    if 1 <= j0 < 3:
                        emit_proj_chunk(j0 - 1)
                    elif j0 == 3:
                        flush_work(1000)
                        emit_proj_chunk(2)
                        emit_proj_head(3, [0, 1, 2])

            # lazy finalize for non-blockwise groups
            def make_finalize(p, g, psY):
                slot = 4 * p + g
                yraw = spool.tile([128, 2, 4, 64], F32, name="yraw", tag="yraw",
                                  bufs=3)
                nc.vector.tensor_copy(yraw[:], psY[:])

                def finalize(p=p, g=g, yraw=yraw, slot=slot):
                    rec = spool.tile([128, 2, 4], F32, name="rec", tag="rec",
                                     bufs=3)
                    nc.vector.reciprocal(rec[:], psDen[:, slot, :, :])
                    y2 = spool.tile([128, 4, 2, 64], BF, name="y2", tag="y2",
                                    bufs=3)
                    # one fused normalize: y2[p,j,h,d] = yraw[p,h,j,d] *
                    # rec[p,h,j] via stride tricks
                    yraw_b = yraw[:].rearrange("p h j d -> p j h d")
                    rec_b = rec[:].rearrange("p h j -> p j h")[
                        :, :, :, None].to_broadcast([128, 4, 2, 64])
                    eng = nc.gpsimd if _Y2_POOL else nc.vector
                    eng.tensor_mul(y2[:], yraw_b, rec_b)
                    for j in range(4):
                        t0 = g * 512 + j * 128
                        nc.sync.dma_start_transpose(yT2_t[:, p, t0:t0 + 128],
                                                    y2[:, j])
                    if g >= 1:
                        nc.gpsimd.tensor_scalar_mul(
                            yT28_t[:, p, (g - 1) * 512:g * 512],
                            yT2_t[:, p, g * 512:(g + 1) * 512], YS)

                return finalize

            # ---------------- emission schedule ----------------
            for t in range(1, TC):
                add_work(t // 4, True, lambda t=t: emit_v_chunk(t))
            for p in range(PAIRS):
                for g in range(TG):
                    if (p, g) == (0, 0):
                        continue
                    add_work(4 * p + g, True,
                             lambda p=p, g=g: emit_qk_group(p, g, "k"), qk=True)
                    add_work(4 * p + g, True,
                             lambda p=p, g=g: emit_qk_group(p, g, "q"), qk=True)

            emit_v_chunk(0)
            emit_qk_group(0, 0, "k")
            emit_qk_group(0, 0, "q")

            pending = None
            for p in range(PAIRS):
                for g in range(TG):
                    row = 4 * p + g
                    flush_work(row)
                    if pending is not None:
                        add_work(row, False, pending, front=True)
                        pending = None
                    if p == PAIRS - 1 and g >= 1:
                        for t in range(4 * (g - 1), 4 * g):
                            for co2 in range(2):
                                add_work(row + 2, True,
                                         lambda t=t, co2=co2:
                                         emit_proj_co2(t, co2))
                    if p == PAIRS - 1 and g == TG - 1:
                        emit_attention_blockwise(p, g)
                        pending = None
                    else:
                        psY = emit_attention_g0(p) if g == 0 \
                            else emit_attention_late(p, g)
                        pending = make_finalize(p, g, psY)

            if pending is not None:
                pending()
            flush_work(100)
            emit_proj_tail(TC - 1)

            if _DEBUG:
                dq = nc.declare_dram_parameter("dbg_q", [128, 512], F32,
                                               isOutput=True)
                dk = nc.declare_dram_parameter("dbg_k", [128, 512], F32,
                                               isOutput=True)
                dv = nc.declare_dram_parameter("dbg_v", [128, 8, 65], F32,
                                               isOutput=True)
                dy = nc.declare_dram_parameter("dbg_yT28", [128, 512], F32,
                                               isOutput=True)
                for src, dst in ((q_t[0][1], dq), (k_t[0][1], dk)):
                    tmp = spool.tile([128, 512], F32, name="dbgt", tag="dbgt",
                                     bufs=2)
                    nc.vector.tensor_copy(tmp[:], src[:])
                    nc.sync.dma_start(dst.ap(), tmp[:])
                tmpv = spool.tile([128, 8, 65], F32, name="dbgv", tag="dbgv")
                nc.vector.tensor_copy(tmpv[:], v8_t[:, 5, :, 0:65])
                nc.sync.dma_start(dv.ap(), tmpv[:])
                tmpy = spool.tile([128, 512], F32, name="dbgy", tag="dbgy")
                nc.vector.tensor_copy(tmpy[:], yT28_t[:, 0, 0:512])
                nc.sync.dma_start(dy.ap(), tmpy[:])

    nc.compile()
    return nc


def _get_compiled():
    global _compiled
    if _compiled is None:
        _compiled = _build()
    return _compiled


def kernel(x, W_attn, W_proj, _trace=False):
    x = np.asarray(x)
    W_attn = np.asarray(W_attn)
    W_proj = np.asarray(W_proj)
    nc = _get_compiled()

    BF16 = ml_dtypes.bfloat16
    FP8NP = ml_dtypes.float8_e4m3
    tri_np = np.triu(np.ones((128, 128), np.float32)).astype(BF16)
    in_maps = []
    for core in range(N_CORES):
        b, hg = core // 2, core % 2
        cols = slice(hg * 512, (hg + 1) * 512)
        xTb = np.ascontiguousarray(x[b].T)
        wqs = W_attn[:, 0 * C:1 * C][:, cols]
        wks = W_attn[:, 1 * C:2 * C][:, cols]
        wvs = W_attn[:, 2 * C:3 * C][:, cols]
        wps = W_proj[hg * 512:(hg + 1) * 512, :]
        in_maps.append({
            "xT": xTb[:, :512].astype(BF16),
            "x8": xTb[:, 512:].astype(FP8NP),
            "wq": wqs.astype(BF16),
            "wk": wks.astype(BF16),
            "wv": wvs.astype(BF16),
            "w8q": (wqs * np.float32(WS)).astype(FP8NP),
            "w8k": (wks * np.float32(WS)).astype(FP8NP),
            "w8v": (wvs * np.float32(WS)).astype(FP8NP),
            "wp": wps.astype(BF16),
            "wp8": (wps * np.float32(WS)).astype(FP8NP),
            "tri": tri_np,
        })

    res = run_bass_kernel_spmd(nc, in_maps, list(range(N_CORES)), trace=_trace)
    out = np.empty((B, T, C), np.float32)
    for b in range(B):
        out[b] = res.results[2 * b]["y"] + res.results[2 * b + 1]["y"]
    if _trace:
        kernel._last_exec_time_ns = res.exec_time_ns
        kernel._last_results = res
    return out


# revision 69
# speedup vs baseline: 1.0206x; 1.0004x over previous
"""Causal self-attention kernel for 8 TRN2 NeuronCores.

Problem: B=4, T=2048, C=1024, H=16 heads, D=64 (fp32 in/out).

Sharding: 8 cores = 4 batch entries x 2 head-groups (8 heads each).
Each core computes, for its (batch b, head-group hg):
    qkv slice -> flash-style causal attention (no-max softmax) -> partial
    projection y_part = attn_out @ W_proj[rows of its heads].
Host sums the two partial projections per batch entry.

Precision strategy (row-split): query rows t<512 (tq-group g=0) follow the
bf16 path everywhere (errors there don't average over keys, so they must
stay accurate); rows t>=512 use fp8e4 DoubleRow matmuls (0.5 cyc/row, 2x
effective contraction) for qkv-gen, att@V and the projection, plus fp8
attention weights produced either by ACT exp (fp8 out, shifted by -SHIFT to
fit e4m3's 240 max) or by a one-op DVE "Schraudolph" exp: psS already holds
S*log2e (k_t is pre-scaled by log2e), so uint8(round(min(psS + S0, 119)))
bit-cast to fp8e4 IS exp(s-SHIFT) to ~3%; uint8 saturation-at-0 provides
the lower clamp. The softmax ratio cancels the shift exactly.

fp8 operands are pre-scaled into e4m3's sweet spot (W by 32, y2 by 4) and
the inverse scales are folded into existing DVE copies, so they are free.

On-device layout:
  - QK^T contracts 64 dims/head (bf16, psS = S*log2e); exp -> pT (g0: bf16;
    g>=1: fp8 in chunk-PAIRED tiles [128, 2, 2h, 512]).
  - att@V g0: per-(chunk,head,block) solo bf16 matmuls (64-col moving);
    g>=1: DoubleRow over chunk pairs (stationary [tk,2,128tq] pT-pair,
    moving [tk,2,64] v-pair), deferred by one pair so the in-order PE queue
    never blocks the next pair's QK^T behind the exp wait.
  - psY is exactly ONE PSUM bank per group ([128,2h,4j,64]); all 16 groups'
    softmax denominators accumulate into a single shared PSUM bank via
    start=False 1-col matmuls (bank memset once; start_tensor_calc would
    wipe the whole bank, so it is never used there).
  - projection: chunks 0-3 bf16 (4-pair contraction); chunks 4-15 DoubleRow
    over pair-pairs from yT2_8 (fp8 copy of the transposed normalized y2).
  - diag masking via tri multiply on DVE (bf16 for g0, fp8 late;
    gpsimd masking loses ~4.7us: Pool queues them behind SWDGE desc-gen).
Software-pipelined emission: deadline-ordered work queue of qkv/proj chunks
interleaved into the ACT/DVE-paced attention loop, as before.
PSUM budget: 2 util + 2x2 psS + 1 psY + 1 den = 8 banks.
TimelineSim: 199.5us (baseline bf16 kernel: 241.5us); rel err 1.08e-2.
"""

import numpy as np
import ml_dtypes
import sys

sys.path.insert(0, "/opt/trn_rl_repo")

import concourse.bass as bass
import concourse.mybir as mybir
import concourse.tile as tile
from concourse import bacc
from concourse.bass_utils import run_bass_kernel_spmd

BF = mybir.dt.bfloat16
F32 = mybir.dt.float32
FP8 = mybir.dt.float8e4
U8 = mybir.dt.uint8
AF = mybir.ActivationFunctionType
DR = mybir.MatmulPerfMode.DoubleRow

B, T, C = 4, 2048, 1024
H, D = 16, 64
N_CORES = 8
HEADS_PER_CORE = 8          # 4 pairs
PAIRS = 4
TC = T // 128               # 16 t-chunks of 128
TG = T // 512               # 4 t-groups of 512
CT = C // 128               # 8 contraction tiles
CIP = CT // 2               # 4 contraction-tile pairs (DoubleRow)
TL = T - 512                # late-row span (t >= 512)

LOG2E = 1.4426950408889634
KAPPA = 1.0 / (8.0 * LOG2E)     # activation scale: exp(psS*KAPPA) = exp(s~)
SHIFT = 3.2                     # p~ = exp(s~ - SHIFT) keeps e4m3 finite
DELTA = 0.32                    # schraudolph interp-error centering
S0 = 56.0 - DELTA - SHIFT * 8.0 * LOG2E
WS = 32.0                       # fp8 weight pre-scale
YS = 4.0                        # fp8 y2 pre-scale
ZINV = 1.0 / (WS * YS)          # late proj PSUM descale

_compiled = None
_DEBUG = False
_EXP_MODE = "mix"   # "act" | "schr" | "mix"
_SCHR_MOD = 8       # in mix mode: chunk is schraudolph iff (c+4g+p)%MOD==MOD-1
_Y2_POOL = False    # fused y2 normalize on gpsimd instead of DVE
_QK_COPY_ACT = 0    # 0=DVE; 1=late k copies on ACT; 2=late q+k copies on ACT


def _build():
    nc = bacc.Bacc("TRN2", target_bir_lowering=False)

    xT = nc.declare_dram_parameter("xT", [C, 512], BF, isOutput=False)
    x8 = nc.declare_dram_parameter("x8", [C, TL], FP8, isOutput=False)
    wq = nc.declare_dram_parameter("wq", [C, 512], BF, isOutput=False)
    wk = nc.declare_dram_parameter("wk", [C, 512], BF, isOutput=False)
    wv = nc.declare_dram_parameter("wv", [C, 512], BF, isOutput=False)
    w8q = nc.declare_dram_parameter("w8q", [C, 512], FP8, isOutput=False)
    w8k = nc.declare_dram_parameter("w8k", [C, 512], FP8, isOutput=False)
    w8v = nc.declare_dram_parameter("w8v", [C, 512], FP8, isOutput=False)
    wp = nc.declare_dram_parameter("wp", [512, C], BF, isOutput=False)
    wp8 = nc.declare_dram_parameter("wp8", [512, C], FP8, isOutput=False)
    tri = nc.declare_dram_parameter("tri", [128, 128], BF, isOutput=False)
    y = nc.declare_dram_parameter("y", [T, C], F32, isOutput=True)

    with tile.TileContext(nc) as tc:
        with (
            tc.tile_pool(name="const", bufs=1) as cpool,
            tc.tile_pool(name="small", bufs=2) as spool,
            tc.tile_pool(name="ps_util", bufs=2, space="PSUM") as ps_util,
            tc.tile_pool(name="ps_s", bufs=2, space="PSUM") as ps_s,
            tc.tile_pool(name="ps_y", bufs=1, space="PSUM") as ps_y,
            tc.tile_pool(name="ps_den", bufs=1, space="PSUM") as ps_den,
        ):
            # softmax denominators for all 16 (pair, group)s live in one
            # shared PSUM bank: zeroed once, then accumulated into by
            # start=False 1-col matmuls (never start_tensor_calc'd, so no
            # bank wipes).  This keeps each psY to exactly one bank.
            psDen = ps_den.tile([128, 16, 2, 4], F32, name="psDen", tag="den")
            nc.vector.memset(psDen[:], 0.0)
            # ---------------- constant loads ----------------
            xT_t = cpool.tile([128, CT, 512], BF, name="xT_t", tag="xT_t")
            x8_t = cpool.tile([128, CT, TL], FP8, name="x8_t", tag="x8_t")
            wv_t = cpool.tile([128, CT, 512], BF, name="wv_t", tag="wv_t")
            wq_t = cpool.tile([128, CT, 512], BF, name="wq_t", tag="wq_t")
            wk_t = cpool.tile([128, CT, 512], BF, name="wk_t", tag="wk_t")
            w8v_t = cpool.tile([128, CIP, 2, 512], FP8, name="w8v_t", tag="w8v_t")
            w8q_t = cpool.tile([128, CIP, 2, 512], FP8, name="w8q_t", tag="w8q_t")
            w8k_t = cpool.tile([128, CIP, 2, 512], FP8, name="w8k_t", tag="w8k_t")
            tri_t = cpool.tile([128, 128], BF, name="tri_t", tag="tri_t")

            xT_r = xT.ap().rearrange("(ct p) t -> p ct t", p=128)
            nc.sync.dma_start(wv_t[:], wv.ap().rearrange("(ct p) d -> p ct d", p=128))
            nc.sync.dma_start(xT_t[:, :, 0:128], xT_r[:, :, 0:128])
            nc.sync.dma_start(xT_t[:, :, 128:512], xT_r[:, :, 128:512])
            nc.sync.dma_start(wk_t[:], wk.ap().rearrange("(ct p) d -> p ct d", p=128))
            nc.sync.dma_start(wq_t[:], wq.ap().rearrange("(ct p) d -> p ct d", p=128))
            nc.sync.dma_start(tri_t[:], tri.ap())
            nc.sync.dma_start(w8v_t[:],
                              w8v.ap().rearrange("(cp s p) d -> p cp s d", p=128, s=2))
            nc.sync.dma_start(x8_t[:], x8.ap().rearrange("(ct p) t -> p ct t", p=128))
            nc.sync.dma_start(w8k_t[:],
                              w8k.ap().rearrange("(cp s p) d -> p cp s d", p=128, s=2))
            nc.sync.dma_start(w8q_t[:],
                              w8q.ap().rearrange("(cp s p) d -> p cp s d", p=128, s=2))
            wp_t = cpool.tile([128, PAIRS, C], BF, name="wp_t", tag="wp_t")
            nc.sync.dma_start(wp_t[:], wp.ap().rearrange("(pr p) co -> p pr co", p=128))
            wp8_t = cpool.tile([128, 2, 2, C], FP8, name="wp8_t", tag="wp8_t")
            nc.sync.dma_start(wp8_t[:],
                              wp8.ap().rearrange("(pp s p) co -> p pp s co", p=128, s=2))

            # activation bias (-SHIFT) for the fp8 exp path
            bias_t = cpool.tile([128, 1], F32, name="bias_t", tag="bias_t")
            nc.vector.memset(bias_t[:], -SHIFT)

            # v tiles: [t-chunk, head, 64 v dims | ones | pad]
            v_t = cpool.tile([128, 4, HEADS_PER_CORE, 66], BF, name="v_t", tag="v_t")
            nc.vector.memset(v_t[:, :, :, 64:65], 1.0)
            v8_t = cpool.tile([128, TC, HEADS_PER_CORE, 66], FP8, name="v8_t",
                              tag="v8_t")
            nc.vector.memset(v8_t[:, :, :, 64:65], 1.0)

            # qT/kT per (pair, group); merged yT2 (+ fp8 copy for late proj)
            q_t = [[cpool.tile([128, 512], BF, name=f"q_{p}_{g}", tag=f"q_{p}_{g}")
                    for g in range(TG)] for p in range(PAIRS)]
            k_t = [[cpool.tile([128, 512], BF, name=f"k_{p}_{g}", tag=f"k_{p}_{g}")
                    for g in range(TG)] for p in range(PAIRS)]
            yT2_t = cpool.tile([128, PAIRS, T], BF, name="yT2", tag="yT2")
            yT28_t = cpool.tile([128, PAIRS, TL], FP8, name="yT28", tag="yT28")

            # ---------------- work-item generators ----------------
            def emit_v_chunk0_head01():
                # v for chunk 0, heads 0-1 only: unblocks (pair0, g0)'s av
                # behind just 0.25MB of wv DMA instead of the full 1MB
                psV = ps_util.tile([128, 128], F32, name="psV_0a", tag="util")
                for ci in range(CT):
                    nc.tensor.matmul(
                        psV[:],
                        xT_t[:, ci, 0:128],
                        wv_t[:, ci, 0:128],
                        start=(ci == 0), stop=(ci == CT - 1),
                    )
                pv = psV[:].rearrange("p (h d) -> p h d", d=64)
                nc.vector.tensor_copy(v_t[:, 0, 0:2, 0:64], pv)
                nc.vector.tensor_copy(v8_t[:, 0, 0:2, 0:64], pv)

            def emit_v_chunk0_rest():
                psV = ps_util.tile([128, 384], F32, name="psV_0b", tag="util")
                for ci in range(CT):
                    nc.tensor.matmul(
                        psV[:],
                        xT_t[:, ci, 0:128],
                        wv_t[:, ci, 128:512],
                        start=(ci == 0), stop=(ci == CT - 1),
                    )
                pv = psV[:].rearrange("p (h d) -> p h d", d=64)
                nc.vector.tensor_copy(v_t[:, 0, 2:8, 0:64], pv)
                nc.vector.tensor_copy(v8_t[:, 0, 2:8, 0:64], pv)

            def emit_v_chunk(tc16):
                psV = ps_util.tile([128, 512], F32, name=f"psV_{tc16}", tag="util")
                if tc16 < 4:
                    for ci in range(CT):
                        nc.tensor.matmul(
                            psV[:],
                            xT_t[:, ci, tc16 * 128:(tc16 + 1) * 128],
                            wv_t[:, ci, :],
                            start=(ci == 0), stop=(ci == CT - 1),
                        )
                    nc.vector.tensor_copy(v_t[:, tc16, :, 0:64], psV[:])
                    nc.vector.tensor_copy(v8_t[:, tc16, :, 0:64], psV[:])
                else:
                    tl = (tc16 - 4) * 128
                    for cp in range(CIP):
                        nc.tensor.matmul(
                            psV[:],
                            x8_t[:, 2 * cp:2 * cp + 2, tl:tl + 128],
                            w8v_t[:, cp, :, :],
                            start=(cp == 0), stop=(cp == CIP - 1),
                            perf_mode=DR,
                        )
                    nc.vector.tensor_scalar_mul(v8_t[:, tc16, :, 0:64], psV[:],
                                                1.0 / WS)

            def emit_qk_group(p, g, which):
                ps = ps_util.tile([128, 512], F32, name=f"ps{which}_{p}_{g}",
                                  tag="util")
                if g == 0:
                    w_t = wq_t if which == "q" else wk_t
                    for ci in range(CT):
                        nc.tensor.matmul(
                            ps[:],
                            w_t[:, ci, p * 128:(p + 1) * 128],
                            xT_t[:, ci, :],
                            start=(ci == 0), stop=(ci == CT - 1),
                        )
                    if which == "q":
                        nc.vector.tensor_copy(q_t[p][0][:], ps[:])
                    else:
                        nc.vector.tensor_scalar_mul(k_t[p][0][:], ps[:], LOG2E)
                else:
                    w8_t = w8q_t if which == "q" else w8k_t
                    tl = (g - 1) * 512
                    for cp in range(CIP):
                        nc.tensor.matmul(
                            ps[:],
                            w8_t[:, cp, :, p * 128:(p + 1) * 128],
                            x8_t[:, 2 * cp:2 * cp + 2, tl:tl + 512],
                            start=(cp == 0), stop=(cp == CIP - 1),
                            perf_mode=DR,
                        )
                    # the psQ->q_t copy can otherwise queue ~2.3us behind
                    # z-muls in DVE's in-order queue at group transitions,
                    # delaying the group's first QK^T; ACT idles right there.
                    if which == "q":
                        if _QK_COPY_ACT >= 2:
                            nc.scalar.activation(q_t[p][g][:], ps[:], AF.Copy,
                                                 scale=1.0 / WS)
                        else:
                            nc.vector.tensor_scalar_mul(q_t[p][g][:], ps[:],
                                                        1.0 / WS)
                    else:
                        if _QK_COPY_ACT >= 1:
                            nc.scalar.activation(k_t[p][g][:], ps[:], AF.Copy,
                                                 scale=LOG2E / WS)
                        else:
                            nc.vector.tensor_scalar_mul(k_t[p][g][:], ps[:],
                                                        LOG2E / WS)

            # ---------------- projection chunk ----------------
            proj_ps = {}

            def _proj_mm_early(psZ, p, tc16, co2, start, stop):
                nc.tensor.matmul(
                    psZ[:],
                    yT2_t[:, p, tc16 * 128:(tc16 + 1) * 128],
                    wp_t[:, p, co2 * 512:(co2 + 1) * 512],
                    start=start, stop=stop,
                )

            def _proj_mm_late(psZ, pp, tc16, co2, start, stop):
                tl = (tc16 - 4) * 128
                nc.tensor.matmul(
                    psZ[:],
                    yT28_t[:, 2 * pp:2 * pp + 2, tl:tl + 128],
                    wp8_t[:, pp, :, co2 * 512:(co2 + 1) * 512],
                    start=start, stop=stop,
                    perf_mode=DR,
                )

            def emit_proj_head(tc16, parts):
                # start the psZ accumulation with a subset of contraction
                # parts (pairs for early chunks, pair-pairs for late)
                tiles = []
                for co2 in range(2):
                    psZ = ps_util.tile([128, 512], F32, name=f"psZ_{tc16}_{co2}",
                                       tag="util")
                    for i, pt in enumerate(parts):
                        if tc16 < 4:
                            _proj_mm_early(psZ, pt, tc16, co2, i == 0, False)
                        else:
                            _proj_mm_late(psZ, pt, tc16, co2, i == 0, False)
                    tiles.append(psZ)
                proj_ps[tc16] = (tiles, parts[-1] + 1)

            def emit_proj_tail(tc16):
                tiles, p0 = proj_ps.pop(tc16)
                nparts = PAIRS if tc16 < 4 else 2
                for co2 in range(2):
                    psZ = tiles[co2]
                    for pt in range(p0, nparts):
                        if tc16 < 4:
                            _proj_mm_early(psZ, pt, tc16, co2, False,
                                           pt == nparts - 1)
                        else:
                            _proj_mm_late(psZ, pt, tc16, co2, False,
                                          pt == nparts - 1)
                    z = spool.tile([128, 512], F32, name="z", tag="z", bufs=8)
                    if tc16 < 4:
                        nc.vector.tensor_copy(z[:], psZ[:])
                    elif tc16 >= 13:
                        nc.scalar.activation(z[:], psZ[:], AF.Copy, scale=ZINV)
                    else:
                        nc.vector.tensor_scalar_mul(z[:], psZ[:], ZINV)
                    eng = nc.gpsimd if (co2 and tc16 < 13) else nc.sync
                    eng.dma_start(
                        y.ap()[tc16 * 128:(tc16 + 1) * 128, co2 * 512:(co2 + 1) * 512],
                        z[:],
                    )

            def emit_proj_chunk(tc16):
                if tc16 < 4:
                    emit_proj_head(tc16, list(range(PAIRS - 1)))
                else:
                    emit_proj_head(tc16, [0])
                emit_proj_tail(tc16)

            def emit_proj_co2(tc16, co2):
                # self-contained half-chunk (one psZ): popped as separate
                # work items so a stalled psZ slot can't wedge the PE queue
                # in front of the next attention chunk's QK^T
                psZ = ps_util.tile([128, 512], F32, name=f"psZc_{tc16}_{co2}",
                                   tag="util")
                nparts = PAIRS if tc16 < 4 else 2
                for pt in range(nparts):
                    if tc16 < 4:
                        _proj_mm_early(psZ, pt, tc16, co2, pt == 0,
                                       pt == nparts - 1)
                    else:
                        _proj_mm_late(psZ, pt, tc16, co2, pt == 0,
                                      pt == nparts - 1)
                z = spool.tile([128, 512], F32, name="z", tag="z", bufs=8)
                if tc16 < 4:
                    nc.vector.tensor_copy(z[:], psZ[:])
                elif tc16 >= 13:
                    nc.scalar.activation(z[:], psZ[:], AF.Copy, scale=ZINV)
                else:
                    nc.vector.tensor_scalar_mul(z[:], psZ[:], ZINV)
                eng = nc.gpsimd if (co2 and tc16 < 13) else nc.sync
                eng.dma_start(
                    y.ap()[tc16 * 128:(tc16 + 1) * 128,
                           co2 * 512:(co2 + 1) * 512],
                    z[:],
                )

            # ---------------- deadline-ordered work queue ----------------
            work = []   # [deadline_row, pe_bearing, fn, done, is_qk]

            def add_work(row, pe, fn, front=False, qk=False):
                it = [row, pe, fn, False, qk]
                if front:
                    work.insert(0, it)
                else:
                    work.append(it)

            def run_item(it):
                it[3] = True
                it[2]()

            def pop_work(row, normal):
                got_pe = False
                for it in work:
                    if it[3] or got_pe:
                        continue
                    if it[0] <= row or (normal and it[0] <= row + 2):
                        run_item(it)
                        got_pe = it[1]
                    elif not normal:
                        break
                while work and work[0][3]:
                    work.pop(0)

            def flush_work(row):
                for it in work:
                    if not it[3] and (it[0] < row or (it[0] == row and it[4])):
                        run_item(it)
                while work and work[0][3]:
                    work.pop(0)

            # ---------------- exp helpers ----------------
            def emit_exp(p, g, c, dest_ap_bf, dest_ap_fp8, psS, jofs):
                """exp of one chunk; g0 -> bf16 on ACT, late -> fp8 on
                ACT (exp) or DVE (schraudolph) chosen per chunk."""
                if g == 0:
                    nc.scalar.activation(dest_ap_bf, psS[:, :, jofs:512],
                                         AF.Exp, scale=KAPPA)
                    return
                use_dve = (c + 4 * g + p) % _SCHR_MOD == _SCHR_MOD - 1 \
                    if _EXP_MODE == "mix" else (_EXP_MODE == "schr")
                if use_dve:
                    nc.vector.tensor_scalar(
                        dest_ap_fp8.bitcast(U8), psS[:, :, jofs:512],
                        S0, 119.0,
                        op0=mybir.AluOpType.add, op1=mybir.AluOpType.min,
                    )
                else:
                    nc.scalar.activation(dest_ap_fp8, psS[:, :, jofs:512],
                                         AF.Exp, scale=KAPPA, bias=bias_t[:, 0:1])

            # ---------------- attention: g == 0 (bf16 path) ----------------
            def emit_attention_g0(p):
                g, row, slot = 0, 4 * p, 4 * p
                psY = ps_y.tile([128, 2, 4, 64], F32, name=f"psY_{p}_0", tag="y")
                started = False
                pending_av = None
                for c in range(4):
                    j0 = c
                    jofs = 128 * j0
                    psS = ps_s.tile([128, 2, 512], F32, name=f"psS_{p}_0_{c}",
                                    tag="s")
                    for h in range(2):
                        nc.tensor.matmul(
                            psS[:, h, jofs:512],
                            k_t[p][0][h * 64:(h + 1) * 64, c * 128:(c + 1) * 128],
                            q_t[p][0][h * 64:(h + 1) * 64, jofs:512],
                            start=True, stop=True,
                        )
                    pT = spool.tile([128, 2, 512], BF, name="pT", tag="pT", bufs=6)
                    emit_exp(p, 0, c, pT[:, :, jofs:512], None, psS, jofs)
                    nc.vector.tensor_mul(
                        pT[:, :, jofs:jofs + 128],
                        pT[:, :, jofs:jofs + 128],
                        tri_t[:, None, :].to_broadcast([128, 2, 128]),
                    )
                    pop_work(row, normal=(c % 2 == 1))

                    def emit_av(c=c, j0=j0, pT=pT):
                        nonlocal started
                        order = list(range(j0 + 1, 4)) + [j0]
                        for h in range(2):
                            for j in order:
                                nc.tensor.matmul(
                                    psY[:, h, j, :],
                                    pT[:, h, 128 * j:128 * (j + 1)],
                                    v_t[:, c, 2 * p + h, 0:64],
                                    start=not started,
                                    stop=(c == 3 and h == 1),
                                    skip_group_check=True,
                                )
                                started = True
                                nc.tensor.matmul(
                                    psDen[:, slot, h, j:j + 1],
                                    pT[:, h, 128 * j:128 * (j + 1)],
                                    v_t[:, c, 2 * p + h, 64:65],
                                    start=False, stop=False,
                                    skip_group_check=True,
                                )

                    # defer av by one chunk (in-order PE queue; see late path)
                    if pending_av is not None:
                        pending_av()
                    pending_av = emit_av
                if pending_av is not None:
                    pending_av()
                return psY

            # ------------- attention: g >= 1 (fp8 DoubleRow path) -------------
            def emit_attention_late(p, g):
                row = 4 * p + g
                slot = 4 * p + g
                npairs = 2 * g + 2
                psY = ps_y.tile([128, 2, 4, 64], F32, name=f"psY_{p}_{g}", tag="y")
                started = False
                pending_av = None
                for m in range(npairs):
                    c0, c1 = 2 * m, 2 * m + 1
                    pT8 = spool.tile([128, 2, 2, 512], FP8, name="pT8", tag="pT8",
                                     bufs=8)
                    for c in (c0, c1):
                        diag = c >= 4 * g
                        j0 = (c - 4 * g) if diag else 0
                        jofs = 128 * j0
                        psS = ps_s.tile([128, 2, 512], F32,
                                        name=f"psS_{p}_{g}_{c}", tag="s")
                        kg, kc = c // 4, c % 4
                        for h in range(2):
                            nc.tensor.matmul(
                                psS[:, h, jofs:512],
                                k_t[p][kg][h * 64:(h + 1) * 64,
                                           kc * 128:(kc + 1) * 128],
                                q_t[p][g][h * 64:(h + 1) * 64, jofs:512],
                                start=True, stop=True,
                            )
                        emit_exp(p, g, c, None, pT8[:, c % 2, :, jofs:512],
                                 psS, jofs)
                        if diag:
                            nc.vector.tensor_mul(
                                pT8[:, c % 2, :, jofs:jofs + 128],
                                pT8[:, c % 2, :, jofs:jofs + 128],
                                tri_t[:, None, :].to_broadcast([128, 2, 128]),
                            )
                        pop_work(row, normal=(c % 2 == 1))

                    # att@V for the pair: DR for blocks valid in both chunks,
                    # solo fp8 for the straddle block (c0 diagonal).  Emission
                    # is DEFERRED by one pair: PE queues are in-order, so an
                    # av emitted right after its own QKs blocks the next
                    # pair's QKs behind the exp wait.
                    def emit_av(m=m, c0=c0, c1=c1, pT8=pT8):
                        nonlocal started
                        j0_solo = (c0 - 4 * g) if c0 >= 4 * g else None
                        j_dr0 = max(0, c1 - 4 * g)
                        last_pair = (m == npairs - 1)
                        for h in range(2):
                            if j0_solo is not None:
                                nc.tensor.matmul(
                                    psY[:, h, j0_solo, :],
                                    pT8[:, 0, h,
                                        128 * j0_solo:128 * (j0_solo + 1)],
                                    v8_t[:, c0, 2 * p + h, 0:64],
                                    start=not started, stop=False,
                                    skip_group_check=True,
                                )
                                started = True
                                nc.tensor.matmul(
                                    psDen[:, slot, h, j0_solo:j0_solo + 1],
                                    pT8[:, 0, h,
                                        128 * j0_solo:128 * (j0_solo + 1)],
                                    v8_t[:, c0, 2 * p + h, 64:65],
                                    start=False, stop=False,
                                    skip_group_check=True,
                                )
                            for j in range(j_dr0, 4):
                                nc.tensor.matmul(
                                    psY[:, h, j, :],
                                    pT8[:, :, h, 128 * j:128 * (j + 1)],
                                    v8_t[:, c0:c0 + 2, 2 * p + h, 0:64],
                                    start=not started,
                                    stop=(last_pair and h == 1 and j == 3),
                                    perf_mode=DR,
                                    skip_group_check=True,
                                )
                                started = True
                                nc.tensor.matmul(
                                    psDen[:, slot, h, j:j + 1],
                                    pT8[:, :, h, 128 * j:128 * (j + 1)],
                                    v8_t[:, c0:c0 + 2, 2 * p + h, 64:65],
                                    start=False, stop=False,
                                    perf_mode=DR,
                                    skip_group_check=True,
                                )

                    if pending_av is not None:
                        pending_av()
                    pending_av = emit_av
                if pending_av is not None:
                    pending_av()
                return psY

            # ---- attention: last (pair, group) blockwise (fp8 solo path) ----
            def finalize_block(p, g, j, psY):
                # normalize straight out of PSUM: psY persists for the whole
                # blockwise group anyway, so no staging copy is needed and the
                # serial finalize->transpose chain is one DVE hop shorter
                slot = 4 * p + g
                recB = spool.tile([128, 2, 1], F32, name="recB", tag="recB",
                                  bufs=4)
                nc.vector.reciprocal(recB[:], psDen[:, slot, :, j:j + 1])
                y2B = spool.tile([128, 2, 64], BF, name="y2B", tag="y2B", bufs=4)
                nc.vector.tensor_mul(
                    y2B[:], psY[:, :, j, :],
                    recB[:, :, 0:1].to_broadcast([128, 2, 64]))
                t0 = g * 512 + j * 128
                nc.scalar.dma_start_transpose(yT2_t[:, p, t0:t0 + 128], y2B[:])
                if g >= 1:
                    nc.gpsimd.tensor_scalar_mul(
                        yT28_t[:, p, t0 - 512:t0 - 384],
                        yT2_t[:, p, t0:t0 + 128], YS)

            def emit_attention_blockwise(p, g):
                row = 4 * p + g
                slot = 4 * p + g
                nchunks = 4 * g + 4
                psY = ps_y.tile([128, 2, 4, 64], F32, name=f"psY_{p}_{g}", tag="y")
                started = False
                for c in range(nchunks):
                    diag = c >= 4 * g
                    j0 = (c - 4 * g) if diag else 0
                    jofs = 128 * j0
                    psS = ps_s.tile([128, 2, 512], F32, name=f"psS_{p}_{g}_{c}",
                                    tag="s")
                    kg, kc = c // 4, c % 4
                    for h in range(2):
                        nc.tensor.matmul(
                            psS[:, h, jofs:512],
                            k_t[p][kg][h * 64:(h + 1) * 64,
                                       kc * 128:(kc + 1) * 128],
                            q_t[p][g][h * 64:(h + 1) * 64, jofs:512],
                            start=True, stop=True,
                        )
                    pT8 = spool.tile([128, 2, 512], FP8, name="pT8b", tag="pT8b",
                                     bufs=8)
                    emit_exp(p, g, c, None, pT8[:, :, jofs:512], psS, jofs)
                    if diag:
                        nc.vector.tensor_mul(
                            pT8[:, :, jofs:jofs + 128],
                            pT8[:, :, jofs:jofs + 128],
                            tri_t[:, None, :].to_broadcast([128, 2, 128]),
                        )
                    pop_work(row, normal=(c % 2 == 1))
                    order = list(range(j0 + 1, 4)) + [j0] if diag else range(4)
                    for h in range(2):
                        for j in order:
                            stop = (c == 4 * g + j)
                            nc.tensor.matmul(
                                psY[:, h, j, :],
                                pT8[:, h, 128 * j:128 * (j + 1)],
                                v8_t[:, c, 2 * p + h, 0:64],
                                start=not started, stop=stop,
                                skip_group_check=True,
                            )
                            started = True
                            nc.tensor.matmul(
                                psDen[:, slot, h, j:j + 1],
                                pT8[:, h, 128 * j:128 * (j + 1)],
                                v8_t[:, c, 2 * p + h, 64:65],
                                start=False, stop=False,
                                skip_group_check=True,
                            )
                    if diag:
                        finalize_block(p, g, j0, psY)
                        if 1 <= j0 < 3:
                            emit_proj_chunk(4 * g + j0 - 1)
                        elif j0 == 3:
                            flush_work(1000)
                            emit_proj_chunk(4 * g + 2)
                            emit_proj_head(4 * g + 3, [0])

            # g0 variant of the blockwise tail: bf16 path, per-block finalize
            # feeding the (cheap, transpose-only) early projection chunks.
            # Run LAST so the expensive g3 projections overlap its attention.
            def emit_attention_g0_blockwise(p, row):
                g, slot = 0, 4 * p
                psY = ps_y.tile([128, 2, 4, 64], F32, name=f"psY_{p}_0b",
                                tag="y")
                started = False
                for c in range(4):
                    j0 = c
                    jofs = 128 * j0
                    psS = ps_s.tile([128, 2, 512], F32, name=f"psS_{p}_0b_{c}",
                                    tag="s")
                    for h in range(2):
                        nc.tensor.matmul(
                            psS[:, h, jofs:512],
                            k_t[p][0][h * 64:(h + 1) * 64, c * 128:(c + 1) * 128],
                            q_t[p][0][h * 64:(h + 1) * 64, jofs:512],
                            start=True, stop=True,
                        )
                    pT = spool.tile([128, 2, 512], BF, name="pT", tag="pT", bufs=6)
                    emit_exp(p, 0, c, pT[:, :, jofs:512], None, psS, jofs)
                    nc.vector.tensor_mul(
                        pT[:, :, jofs:jofs + 128],
                        pT[:, :, jofs:jofs + 128],
                        tri_t[:, None, :].to_broadcast([128, 2, 128]),
                    )
                    pop_work(row, normal=(c % 2 == 1))
                    order = list(range(j0 + 1, 4)) + [j0]
                    for h in range(2):
                        for j in order:
                            nc.tensor.matmul(
                                psY[:, h, j, :],
                                pT[:, h, 128 * j:128 * (j + 1)],
                                v_t[:, c, 2 * p + h, 0:64],
                                start=not started, stop=(c == j),
                                skip_group_check=True,
                            )
                            started = True
                            nc.tensor.matmul(
                                psDen[:, slot, h, j:j + 1],
                                pT[:, h, 128 * j:128 * (j + 1)],
                                v_t[:, c, 2 * p + h, 64:65],
                                start=False, stop=False,
                                skip_group_check=True,
                            )
                    finalize_block(p, 0, j0, psY)
                    if 1 <= j0 < 3:
                        emit_proj_chunk(j0 - 1)
                    elif j0 == 3:
                        flush_work(1000)
                        emit_proj_chunk(2)
                        emit_proj_head(3, [0, 1, 2])

            # lazy finalize for non-blockwise groups
            def make_finalize(p, g, psY):
                slot = 4 * p + g
                yraw = spool.tile([128, 2, 4, 64], F32, name="yraw", tag="yraw",
                                  bufs=4)
                nc.vector.tensor_copy(yraw[:], psY[:])

                def finalize(p=p, g=g, yraw=yraw, slot=slot):
                    rec = spool.tile([128, 2, 4], F32, name="rec", tag="rec",
                                     bufs=4)
                    nc.vector.reciprocal(rec[:], psDen[:, slot, :, :])
                    y2 = spool.tile([128, 4, 2, 64], BF, name="y2", tag="y2",
                                    bufs=4)
                    # one fused normalize: y2[p,j,h,d] = yraw[p,h,j,d] *
                    # rec[p,h,j] via stride tricks
                    yraw_b = yraw[:].rearrange("p h j d -> p j h d")
                    rec_b = rec[:].rearrange("p h j -> p j h")[
                        :, :, :, None].to_broadcast([128, 4, 2, 64])
                    eng = nc.gpsimd if _Y2_POOL else nc.vector
                    eng.tensor_mul(y2[:], yraw_b, rec_b)
                    for j in range(4):
                        t0 = g * 512 + j * 128
                        nc.sync.dma_start_transpose(yT2_t[:, p, t0:t0 + 128],
                                                    y2[:, j])
                    if g >= 1:
                        nc.gpsimd.tensor_scalar_mul(
                            yT28_t[:, p, (g - 1) * 512:g * 512],
                            yT2_t[:, p, g * 512:(g + 1) * 512], YS)

                return finalize

            # ---------------- emission schedule ----------------
            for t in range(1, TC):
                add_work(t // 4, True, lambda t=t: emit_v_chunk(t))
            for p in range(PAIRS):
                for g in range(TG):
                    if (p, g) == (0, 0):
                        continue
                    add_work(4 * p + g, True,
                             lambda p=p, g=g: emit_qk_group(p, g, "k"), qk=True)
                    add_work(4 * p + g, True,
                             lambda p=p, g=g: emit_qk_group(p, g, "q"), qk=True)

            emit_v_chunk(0)
            emit_qk_group(0, 0, "k")
            emit_qk_group(0, 0, "q")

            pending = None
            for p in range(PAIRS):
                for g in range(TG):
                    row = 4 * p + g
                    flush_work(row)
                    if pending is not None:
                        add_work(row, False, pending, front=True)
                        pending = None
                    if p == PAIRS - 1 and g >= 1:
                        for t in range(4 * (g - 1), 4 * g):
                            for co2 in range(2):
                                add_work(row + 2, True,
                                         lambda t=t, co2=co2:
                                         emit_proj_co2(t, co2))
                    if p == PAIRS - 1 and g == TG - 1:
                        emit_attention_blockwise(p, g)
                        pending = None
                    else:
                        psY = emit_attention_g0(p) if g == 0 \
                            else emit_attention_late(p, g)
                        pending = make_finalize(p, g, psY)

            if pending is not None:
                pending()
            flush_work(100)
            emit_proj_tail(TC - 1)

            if _DEBUG:
                dq = nc.declare_dram_parameter("dbg_q", [128, 512], F32,
                                               isOutput=True)
                dk = nc.declare_dram_parameter("dbg_k", [128, 512], F32,
                                               isOutput=True)
                dv = nc.declare_dram_parameter("dbg_v", [128, 8, 65], F32,
                                               isOutput=True)
                dy = nc.declare_dram_parameter("dbg_yT28", [128, 512], F32,
                                               isOutput=True)
                for src, dst in ((q_t[0][1], dq), (k_t[0][1], dk)):
                    tmp = spool.tile([128, 512], F32, name="dbgt", tag="dbgt",
                                     bufs=2)
                    nc.vector.tensor_copy(tmp[:], src[:])
                    nc.sync.dma_start(dst.ap(), tmp[:])
                tmpv = spool.tile([128, 8, 65], F32, name="dbgv", tag="dbgv")
                nc.vector.tensor_copy(tmpv[:], v8_t[:, 5, :, 0:65])
                nc.sync.dma_start(dv.ap(), tmpv[:])
                tmpy = spool.tile([128, 512], F32, name="dbgy", tag="dbgy")
                nc.vector.tensor_copy(tmpy[:], yT28_t[:, 0, 0:512])
                nc.sync.dma_start(dy.ap(), tmpy[:])

    nc.compile()
    return nc


def _get_compiled():
    global _compiled
    if _compiled is None:
        _compiled = _build()
    return _compiled


def kernel(x, W_attn, W_proj, _trace=False):
    x = np.asarray(x)
    W_attn = np.asarray(W_attn)
    W_proj = np.asarray(W_proj)
    nc = _get_compiled()

    BF16 = ml_dtypes.bfloat16
    FP8NP = ml_dtypes.float8_e4m3
    tri_np = np.triu(np.ones((128, 128), np.float32)).astype(BF16)
    in_maps = []
    for core in range(N_CORES):
        b, hg = core // 2, core % 2
        cols = slice(hg * 512, (hg + 1) * 512)
        xTb = np.ascontiguousarray(x[b].T)
        wqs = W_attn[:, 0 * C:1 * C][:, cols]
        wks = W_attn[:, 1 * C:2 * C][:, cols]
        wvs = W_attn[:, 2 * C:3 * C][:, cols]
        wps = W_proj[hg * 512:(hg + 1) * 512, :]
        in_maps.append({
            "xT": xTb[:, :512].astype(BF16),
            "x8": xTb[:, 512:].astype(FP8NP),
            "wq": wqs.astype(BF16),
            "wk": wks.astype(BF16),
            "wv": wvs.astype(BF16),
            "w8q": (wqs * np.float32(WS)).astype(FP8NP),
            "w8k": (wks * np.float32(WS)).astype(FP8NP),
            "w8v": (wvs * np.float32(WS)).astype(FP8NP),
            "wp": wps.astype(BF16),
            "wp8": (wps * np.float32(WS)).astype(FP8NP),
            "tri": tri_np,
        })

    res = run_bass_kernel_spmd(nc, in_maps, list(range(N_CORES)), trace=_trace)
    out = np.empty((B, T, C), np.float32)
    for b in range(B):
        out[b] = res.results[2 * b]["y"] + res.results[2 * b + 1]["y"]
    if _trace:
        kernel._last_exec_time_ns = res.exec_time_ns
        kernel._last_results = res
    return out
